# revision 1
# baseline (speedup 1.0000x reference)
"""Trainium2 Bass kernel for a DiT-style transformer block (adaLN modulation,
RoPE self-attention with additive rank mask, hybrid cross-attention to
[clean|observed] memory, gated MLP).

Sharding: 8 cores = 4 batches x 2 sequence-halves. Each core computes the
block output for its 512 query tokens of one batch. Per-core token order is
permuted (host side) so the core's own tokens come first, which keeps the
program SPMD-static across cores.

Layout: activations live feature-major ("T-layout", [feature, token]) so all
matmuls contract along partitions. Matmul operands use dtype float32r
(full-rate PE, ~1.5e-4 rms rel error vs fp32). Softmax runs without
max-subtraction (scores are O(10)); masking multiplies probabilities by
exp(mask) in {0,1}. Softmax denominators come free from a ones-column
appended to each head's value block (p@v output row 64). The memory layernorm
is folded through the KV projection (per-token affine commutes with the
feature contraction): kv = rs_t*(W@mem) - (mu*rs)_t*rowsum(W).
"""

import numpy as np
from contextlib import ExitStack

from concourse import bacc, mybir
import concourse.bass as bass
import concourse.tile as tile
from concourse import bass_utils

F32 = mybir.dt.float32
F32R = mybir.dt.float32r
AF = mybir.ActivationFunctionType
OP = mybir.AluOpType

P = 128


class Cfg:
    def __init__(self, mini=False):
        if mini:
            self.B, self.N, self.D, self.H, self.HD = 2, 256, 256, 4, 64
            self.COND = 128
        else:
            self.B, self.N, self.D, self.H, self.HD = 4, 1024, 1024, 16, 64
            self.COND = 256
        self.DH = 4 * self.D
        self.SQ = self.N // 2            # own query tokens per core
        self.CH = self.D // P            # d-chunks
        self.HH = self.H * self.HD // P  # head-pair chunks (= H // 2)
        self.KK = self.N // P            # key chunks per N tokens
        self.NF = self.N // self.SQ      # token-free blocks of SQ (=2)
        self.CC = self.COND // P
        self.DHC = self.DH // P
        self.QKK = self.SQ // P          # key chunks per memory quarter
        self.n_cores = 2 * self.B
        self.eps = 1e-5


def _dma_bcast(nc, out_tile, dram_ap, off, n):
    """DMA dram row [1, off:off+n] broadcast to all partitions [P, n]."""
    src = bass.AP(
        tensor=dram_ap.tensor, offset=dram_ap.offset + off, ap=[[0, P], [1, n]]
    )
    nc.gpsimd.dma_start(out=out_tile, in_=src)


def _shift32_dma(nc, dst, src):
    """dst[p] = src[p xor-32 within each 64-block], [128, F] SBUF tiles."""
    for blk in range(2):
        b = blk * 64
        nc.sync.dma_start(out=dst[b : b + 32, :], in_=src[b + 32 : b + 64, :])
        nc.sync.dma_start(out=dst[b + 32 : b + 64, :], in_=src[b : b + 32, :])


def r(ap):
    """fp32 view of an f32r AP for DVE/ACT/gpsimd input reads."""
    return ap.bitcast(F32)


def build_program(cfg: Cfg):
    c = cfg
    nc = bacc.Bacc(
        "TRN2",
        target_bir_lowering=False,
        debug=False,
        enable_asserts=True,
        num_devices=c.n_cores,
    )

    def din(name, shape, dt=F32R):
        return nc.dram_tensor(name, shape, dt, kind="ExternalInput").ap()

    xT = din("xT", [c.D, c.N])
    tcT = din("tcT", [c.COND, c.N])
    hcT = din("hcT", [c.D, c.N])
    hoT = din("hoT", [c.D, c.N])
    wadaT = din("wadaT", [c.COND, 9 * c.D])
    wqkvT = din("wqkvT", [c.D, 3 * c.D])
    wselfT = din("wselfT", [c.D, c.D])
    wqT = din("wqT", [c.D, c.D])
    wkvT = din("wkvT", [c.D, 2 * c.D])
    wcrossT = din("wcrossT", [c.D, c.D])
    wm1T = din("wm1T", [c.D, c.DH])
    wm2T = din("wm2T", [c.DH, c.D])
    bada = din("bada", [P, 9 * c.CH], F32)
    bm1 = din("bm1", [P, c.DHC], F32)
    bm2 = din("bm2", [P, c.CH], F32)
    cqt = din("cqt", [P, c.SQ], F32)
    sqt = din("sqt", [P, c.SQ], F32)
    ckts = din("ckts", [P, c.N], F32)
    skts = din("skts", [P, c.N], F32)
    cktm = din("cktm", [P, c.N], F32)
    sktm = din("sktm", [P, c.N], F32)
    mself = din("mself", [c.N, c.SQ], F32)
    mhc = din("mhc", [c.N, c.SQ], F32)
    mho = din("mho", [c.N, c.SQ], F32)
    la_self = din("la_self", [1, c.N], F32)   # rstd per own-order token
    lb_self = din("lb_self", [1, c.N], F32)   # mean per own-order token
    la_mc = din("la_mc", [1, c.N], F32)       # rstd, clean memory
    lb_mc = din("lb_mc", [1, c.N], F32)       # mean*rstd, clean memory
    la_mo = din("la_mo", [1, c.N], F32)
    lb_mo = din("lb_mo", [1, c.N], F32)
    swk = din("swk", [P, c.HH], F32)          # rowsum(Wk) per k-feature
    wsumv = din("wsumv", [1, c.H * c.HD], F32)  # rowsum(Wv) per v-feature
    rs_cols = din("rs_cols", [P, 2 * c.KK], F32)    # mem rstd, column layout
    mrs_cols = din("mrs_cols", [P, 2 * c.KK], F32)  # mem mean*rstd, columns
    out_d = nc.dram_tensor("out", [c.D, c.SQ], F32, kind="ExternalOutput").ap()
    xc_d = nc.dram_tensor("xc_scratch", [c.D, c.SQ], F32R, kind="Internal").ap()
    xc2_d = nc.dram_tensor("xc2_scratch", [c.D, c.SQ], F32R, kind="Internal").ap()

    with ExitStack() as ctx:
        tc = ctx.enter_context(tile.TileContext(nc))
        persist = ctx.enter_context(tc.tile_pool(name="persist", bufs=1))
        ws = ctx.enter_context(tc.tile_pool(name="wstream", bufs=1))
        tw_pool = ctx.enter_context(tc.tile_pool(name="tw", bufs=6))
        rsp = ctx.enter_context(tc.tile_pool(name="rsp", bufs=1))
        small = ctx.enter_context(tc.tile_pool(name="small", bufs=1))

        def wtile():
            return ws.tile([P, P], F32R, name="wt", tag="wt", bufs=8)

        def wbtile(nk):
            return ws.tile([P, nk, P], F32R, name=f"wb{nk}", tag=f"wb{nk}",
                           bufs=3)


        def tw():
            return tw_pool.tile([P, c.SQ], F32, name="tw", tag="tw")

        # ---------- persistent preloads ----------
        TC = persist.tile([P, c.CC, c.N], F32R)
        nc.sync.dma_start(out=TC, in_=tcT.rearrange("(k p) n -> p k n", p=P))
        CQ = persist.tile([P, c.SQ], F32)
        nc.sync.dma_start(out=CQ, in_=cqt)
        SQt = persist.tile([P, c.SQ], F32)
        nc.sync.dma_start(out=SQt, in_=sqt)
        BADA = persist.tile([P, 9 * c.CH], F32)
        nc.sync.dma_start(out=BADA, in_=bada)
        BM1 = persist.tile([P, c.DHC], F32)
        nc.sync.dma_start(out=BM1, in_=bm1)
        BM2 = persist.tile([P, c.CH], F32)
        nc.sync.dma_start(out=BM2, in_=bm2)

        EPS = persist.tile([P, 1], F32)
        nc.vector.memset(EPS, 1e-5)
        ones_f32 = persist.tile([P, 16], F32)
        nc.vector.memset(ones_f32, 1.0)
        ONE = persist.tile([P, 1], F32R)
        nc.vector.tensor_copy(ONE, ones_f32[:, 0:1])
        ONES16 = persist.tile([P, 16], F32R)
        nc.vector.tensor_copy(ONES16, ones_f32)

        # ---------- helpers ----------
        def ada_modulate(q_sh, q_sc, x_src, x_nf, la_b, lb_b, xn_dst):
            """xn = x*sc1 - m*sc1 + sh, with sc1 = rs*w*(1+sc).

            la_b(cols) -> [P, SQ] rstd broadcast AP; lb_b(cols) -> mean.
            x_src(j, tf) / xn_dst(j, tf): [P, SQ] APs.
            """
            with tc.tile_pool(name="ps_ada", bufs=1, space="PSUM") as psa:
                for j in range(c.CH):
                    ps_sh = [
                        psa.tile([P, c.SQ], F32, name=f"ps_sh{t}", tag=f"ps_sh{t}")
                        for t in range(x_nf)
                    ]
                    ps_sc = [
                        psa.tile([P, c.SQ], F32, name=f"ps_sc{t}", tag=f"ps_sc{t}")
                        for t in range(x_nf)
                    ]
                    wt = wbtile(c.CC)
                    nc.sync.dma_start(
                        out=wt,
                        in_=wadaT[
                            :, q_sh * c.D + j * P : q_sh * c.D + (j + 1) * P
                        ].rearrange("(k p) m -> p k m", p=P),
                    )
                    wt2 = wbtile(c.CC)
                    nc.sync.dma_start(
                        out=wt2,
                        in_=wadaT[
                            :, q_sc * c.D + j * P : q_sc * c.D + (j + 1) * P
                        ].rearrange("(k p) m -> p k m", p=P),
                    )
                    for k in range(c.CC):
                        for tf in range(x_nf):
                            nc.tensor.matmul(
                                ps_sh[tf], wt[:, k, :],
                                TC[:, k, tf * c.SQ : (tf + 1) * c.SQ],
                                start=(k == 0), stop=(k == c.CC - 1),
                            )
                        for tf in range(x_nf):
                            nc.tensor.matmul(
                                ps_sc[tf], wt2[:, k, :],
                                TC[:, k, tf * c.SQ : (tf + 1) * c.SQ],
                                start=(k == 0), stop=(k == c.CC - 1),
                            )
                    for tf in range(x_nf):
                        cols = slice(tf * c.SQ, (tf + 1) * c.SQ)
                        sc1 = tw()
                        nc.vector.scalar_tensor_tensor(
                            out=sc1, in0=ps_sc[tf],
                            scalar=BADA[:, q_sc * c.CH + j : q_sc * c.CH + j + 1],
                            in1=la_b(cols), op0=OP.add, op1=OP.mult,
                        )
                        mm = tw()
                        nc.vector.tensor_mul(mm, lb_b(cols), sc1)
                        sh = tw()
                        nc.vector.scalar_tensor_tensor(
                            out=sh, in0=ps_sh[tf],
                            scalar=BADA[:, q_sh * c.CH + j : q_sh * c.CH + j + 1],
                            in1=mm, op0=OP.add, op1=OP.subtract,
                        )
                        t = tw()
                        nc.vector.tensor_mul(t, x_src(j, tf), sc1)
                        nc.vector.tensor_add(xn_dst(j, tf), t, sh)

        def ada_gate_one(q_g, j, psg):
            """Return a [P, SQ] f32 tile holding gate chunk j on demand."""
            ps = psg.tile([P, c.SQ], F32, name="ps_g", tag="ps_g")
            wt = wbtile(c.CC)
            nc.sync.dma_start(
                out=wt,
                in_=wadaT[
                    :, q_g * c.D + j * P : q_g * c.D + (j + 1) * P
                ].rearrange("(k p) m -> p k m", p=P),
            )
            for k in range(c.CC):
                nc.tensor.matmul(
                    ps, wt[:, k, :], TC[:, k, 0 : c.SQ],
                    start=(k == 0), stop=(k == c.CC - 1),
                )
            g = tw()
            nc.vector.tensor_scalar_add(
                g, ps, BADA[:, q_g * c.CH + j : q_g * c.CH + j + 1]
            )
            return g

        def rope_evict(zsrc, hh, cols_t, ctab, stab, dst):
            """dst[:, hh, cols_t] = zsrc*cos + shift32(zsrc)*sin_signed."""
            t1 = tw()
            nc.vector.tensor_mul(t1, zsrc, ctab)
            tsh = tw()
            _shift32_dma(nc, tsh, zsrc)
            nc.vector.tensor_mul(tsh, tsh, stab)
            nc.vector.tensor_add(dst[:, hh, cols_t], t1, tsh)

        def proj_rope(wT_dram, col_off, n_free, ctab, stab, dst, src_tile):
            """dst[:, hh, :] = rope(W[:, cols].T @ src), head-pair chunks."""
            nf = n_free // c.SQ
            with tc.tile_pool(name="ps_qk", bufs=3, space="PSUM") as psq:
                for hh in range(c.HH):
                    wt = wbtile(c.CH)
                    nc.sync.dma_start(
                        out=wt,
                        in_=wT_dram[
                            :, col_off + hh * P : col_off + (hh + 1) * P
                        ].rearrange("(k p) m -> p k m", p=P),
                    )
                    for tf in range(nf):
                        ps = psq.tile([P, c.SQ], F32, name="ps_qk", tag="ps_qk")
                        for k in range(c.CH):
                            nc.tensor.matmul(
                                ps, wt[:, k, :],
                                src_tile[:, k, tf * c.SQ : (tf + 1) * c.SQ],
                                start=(k == 0), stop=(k == c.CH - 1),
                            )
                        cols = slice(tf * c.SQ, (tf + 1) * c.SQ)
                        traw = tw()
                        nc.scalar.activation(traw, ps, AF.Copy)
                        rope_evict(
                            traw, hh, cols, ctab[:, cols], stab[:, cols], dst
                        )

        def vproj_self(src_tile, vdst, wvp):
            """Token-major value projection from resident XN; ones cols."""
            ntt = c.KK
            ffw = min(512, c.H * c.HD)
            nff = (c.H * c.HD) // ffw
            hpf = ffw // 64
            for tt in range(ntt):
                ap = vdst[:, tt, :].rearrange("p (h e) -> p h e", e=65)[:, :, 64:65]
                nc.vector.tensor_copy(ap, ONES16[:, 0 : c.H])
            with tc.tile_pool(name="ps_v", bufs=8, space="PSUM") as psv:
                for ff in range(nff):
                    pss = [
                        psv.tile([P, ffw], F32, name="ps_v", tag="ps_v")
                        for _ in range(ntt)
                    ]
                    kh = max(1, c.CH // 4)
                    for kg in range(c.CH // kh):
                        wt = wvp.tile([P, kh, ffw], F32R, name="wv", tag="wv",
                                      bufs=2)
                        nc.sync.dma_start(
                            out=wt,
                            in_=wqkvT[
                                kg * kh * P : (kg + 1) * kh * P,
                                2 * c.D + ff * ffw : 2 * c.D + (ff + 1) * ffw,
                            ].rearrange("(k p) m -> p k m", p=P),
                        )
                        for k in range(kh):
                            gk = kg * kh + k
                            for tt in range(ntt):
                                nc.tensor.matmul(
                                    pss[tt],
                                    src_tile[:, gk, tt * P : (tt + 1) * P],
                                    wt[:, k, :],
                                    start=(gk == 0), stop=(gk == c.CH - 1),
                                )
                    for tt in range(ntt):
                        ap = (
                            vdst[:, tt, ff * hpf * 65 : (ff + 1) * hpf * 65]
                            .rearrange("p (h e) -> p h e", e=65)[:, :, 0:64]
                        )
                        nc.vector.tensor_copy(ap, pss[tt])

        def attention_hp(hp, khat, vtile, qhat, masks, n_kk, ps_o1, ps_o2,
                         tp_pool, first, last):
            """One head pair, software-pipelined: p@v lags scores by one
            chunk so the PE has independent work while ACT/DVE/GpSimd chew
            through exp+mask of the current chunk."""
            h1, h2 = 2 * hp, 2 * hp + 1

            def pv(kkc, pt):
                nc.tensor.matmul(
                    ps_o1, vtile[:, kkc, h1 * 65 : (h1 + 1) * 65],
                    pt[:, 0 : c.SQ],
                    start=(first and kkc == 0),
                    stop=(last and kkc == n_kk - 1),
                )
                nc.tensor.matmul(
                    ps_o2, vtile[:, kkc, h2 * 65 : (h2 + 1) * 65],
                    pt[:, c.SQ : 2 * c.SQ],
                    start=(first and kkc == 0),
                    stop=(last and kkc == n_kk - 1),
                )

            with tc.tile_pool(name="ps_s", bufs=2, space="PSUM") as pss:
                prev = None
                for kkc in range(n_kk):
                    ps = pss.tile([P, 2 * c.SQ], F32, name="ps_s", tag="ps_s")
                    ks = slice(kkc * P, (kkc + 1) * P)
                    nc.tensor.matmul(
                        ps[:, 0 : c.SQ],
                        khat[0:64, hp, ks], qhat[0:64, hp, :],
                        start=True, stop=True,
                    )
                    nc.tensor.matmul(
                        ps[:, c.SQ : 2 * c.SQ],
                        khat[64:128, hp, ks], qhat[64:128, hp, :],
                        start=True, stop=True,
                    )
                    pt = tp_pool.tile(
                        [P, 2 * c.SQ], F32R, name="t_p", tag="t_p", bufs=3
                    )
                    nc.scalar.activation(pt, ps, AF.Exp)
                    nc.vector.tensor_mul(
                        pt[:, 0 : c.SQ], r(pt[:, 0 : c.SQ]), masks[:, kkc, :]
                    )
                    eng2 = nc.vector if (kkc % 3 == 2) else nc.gpsimd
                    eng2.tensor_mul(
                        pt[:, c.SQ : 2 * c.SQ], r(pt[:, c.SQ : 2 * c.SQ]),
                        masks[:, kkc, :],
                    )
                    if prev is not None:
                        pv(*prev)
                    prev = (kkc, pt)
                pv(*prev)

        def evict_unnorm(ps_o, hp, second, odst, den, tp_pool):
            """Stage unnormalized o rows into odst and the denominator row
            into den[2hp+second]. Normalization happens batched later."""
            h = 2 * hp + (1 if second else 0)
            dstage = tp_pool.tile(
                [65, c.SQ], F32, name="t_dstage", tag="t_dstage", bufs=2
            )
            nc.vector.tensor_copy(dstage[64:65, :], ps_o[64:65, :])
            nc.sync.dma_start(out=den[h : h + 1, :], in_=dstage[64:65, :])
            if not second:
                nc.vector.tensor_copy(odst[0:64, hp, :], ps_o[0:64, :])
            else:
                st = tp_pool.tile(
                    [64, c.SQ], F32R, name="t_onorm", tag="t_onorm", bufs=2
                )
                nc.vector.tensor_copy(st, ps_o[0:64, :])
                nc.sync.dma_start(out=odst[64:128, hp, :], in_=st)

        def normalize_batch(odst, den, deni, tp_pool, n_hp):
            """odst[:, hp, :] *= 1/den rows (broadcast per head)."""
            nc.vector.reciprocal(deni, den)
            for hp in range(n_hp):
                d1 = small.tile([1, c.SQ], F32, name="s_d1", tag="s_d1",
                                bufs=2)
                nc.sync.dma_start(out=d1, in_=deni[2 * hp : 2 * hp + 1, :])
                d2 = small.tile([1, c.SQ], F32, name="s_d2", tag="s_d2",
                                bufs=2)
                nc.sync.dma_start(out=d2, in_=deni[2 * hp + 1 : 2 * hp + 2, :])
                rb = tp_pool.tile(
                    [P, c.SQ], F32, name="t_rb", tag="t_rb", bufs=2
                )
                nc.gpsimd.partition_broadcast(rb[0:64, :], d1, channels=64)
                rh = tp_pool.tile(
                    [64, c.SQ], F32, name="t_rh", tag="t_rh", bufs=2
                )
                nc.gpsimd.partition_broadcast(rh, d2, channels=64)
                nc.sync.dma_start(out=rb[64:128, :], in_=rh)
                nc.vector.tensor_mul(
                    odst[:, hp, :], r(odst[:, hp, :]), rb
                )

        def out_proj_residual(wT_dram, osrc, g_src, xr, xdst_dram):
            with tc.tile_pool(name="ps_op", bufs=3, space="PSUM") as pso:
                for j in range(c.CH):
                    ps = pso.tile([P, c.SQ], F32, name="ps_op", tag="ps_op")
                    wt = wbtile(c.HH)
                    nc.sync.dma_start(
                        out=wt,
                        in_=wT_dram[:, j * P : (j + 1) * P].rearrange(
                            "(k p) m -> p k m", p=P
                        ),
                    )
                    for hp in range(c.HH):
                        nc.tensor.matmul(
                            ps, wt[:, hp, :], osrc[:, hp, :],
                            start=(hp == 0), stop=(hp == c.HH - 1),
                        )
                    t = tw()
                    nc.vector.tensor_mul(t, ps, g_src(j))
                    t2 = tw()
                    nc.vector.tensor_add(t2, t, xr(j))
                    nc.sync.dma_start(
                        out=xdst_dram[j * P : (j + 1) * P, :], in_=t2.bitcast(F32R)
                    )

        def device_ln_stats(x_src):
            """[P, SQ] broadcast (rstd, mean) tiles; x_src(j) -> f32r AP."""
            rs_b = rsp.tile([P, c.SQ], F32, name="t_rsb", tag="t_rsb")
            m_b = rsp.tile([P, c.SQ], F32, name="t_mb", tag="t_mb")
            with tc.tile_pool(name="ps_st", bufs=1, space="PSUM") as psst, \
                 tc.tile_pool(name="stats", bufs=1) as stp:
                ps1 = psst.tile([1, c.SQ], F32, name="ps_st1", tag="ps_st1")
                ps2 = psst.tile([1, c.SQ], F32, name="ps_st2", tag="ps_st2")
                for j in range(c.CH):
                    xa = x_src(j)
                    sq = stp.tile([P, c.SQ], F32R, name="t_sq", tag="t_sq",
                                  bufs=2)
                    nc.vector.tensor_mul(sq, r(xa), r(xa))
                    nc.tensor.matmul(
                        ps1, ONE, xa, start=(j == 0), stop=(j == c.CH - 1)
                    )
                    nc.tensor.matmul(
                        ps2, ONE, sq, start=(j == 0), stop=(j == c.CH - 1)
                    )
                m = stp.tile([1, c.SQ], F32, name="s_m", tag="s_m")
                nc.vector.tensor_scalar_mul(m, ps1[0:1, :], 1.0 / c.D)
                e2 = stp.tile([1, c.SQ], F32, name="s_a", tag="s_a")
                nc.vector.tensor_scalar_mul(e2, ps2[0:1, :], 1.0 / c.D)
                msq = stp.tile([1, c.SQ], F32, name="s_b", tag="s_b")
                nc.vector.tensor_mul(msq, m, m)
                var = stp.tile([1, c.SQ], F32, name="s_c", tag="s_c")
                nc.vector.tensor_sub(var, e2, msq)
                sd = stp.tile([1, c.SQ], F32, name="s_d", tag="s_d")
                nc.scalar.activation(sd, var, AF.Sqrt, bias=EPS[0:1, :])
                rs = stp.tile([1, c.SQ], F32, name="s_e", tag="s_e")
                nc.vector.reciprocal(rs, sd)
                nc.gpsimd.partition_broadcast(rs_b, rs, channels=P)
                nc.gpsimd.partition_broadcast(m_b, m, channels=P)
            return rs_b, m_b

        def stream_x(dram, j, cols):
            t = tw()
            nc.sync.dma_start(out=t, in_=r(dram[j * P : (j + 1) * P, cols]))
            return t

        def stream_xr(dram, j):
            t = tw_pool.tile([P, c.SQ], F32R, name="twr", tag="twr", bufs=2)
            nc.sync.dma_start(out=t, in_=dram[j * P : (j + 1) * P, :])
            return t

        # =======================================================
        # Phase 1: self-attention
        # =======================================================
        with tc.tile_pool(name="p1", bufs=1) as p1:
            QHAT = p1.tile([P, c.HH, c.SQ], F32R)
            KHAT = p1.tile([P, c.HH, c.N], F32R)
            VSELF = p1.tile([P, c.KK, c.H * 65], F32R)

            with tc.tile_pool(name="p1a", bufs=1) as p1a:
                XN = p1a.tile([P, c.CH, c.N], F32R)
                CKs_t = p1a.tile([P, c.N], F32)
                nc.sync.dma_start(out=CKs_t, in_=ckts)
                SKs_t = p1a.tile([P, c.N], F32)
                nc.sync.dma_start(out=SKs_t, in_=skts)
                with tc.tile_pool(name="p1ln", bufs=1) as p1ln:
                    LAs = p1ln.tile([P, c.N], F32)
                    _dma_bcast(nc, LAs, la_self, 0, c.N)
                    LBs = p1ln.tile([P, c.N], F32)
                    _dma_bcast(nc, LBs, lb_self, 0, c.N)
                    ada_modulate(
                        0, 1,
                        lambda j, tf: stream_x(
                            xT, j, slice(tf * c.SQ, (tf + 1) * c.SQ)
                        ),
                        c.NF,
                        lambda cols: LAs[:, cols],
                        lambda cols: LBs[:, cols],
                        lambda j, tf: XN[:, j, tf * c.SQ : (tf + 1) * c.SQ],
                    )
                proj_rope(wqkvT, 0, c.SQ, CQ, SQt, QHAT, XN)
                proj_rope(wqkvT, c.D, c.N, CKs_t, SKs_t, KHAT, XN)
                with tc.tile_pool(name="wvp1", bufs=1) as wvp:
                    vproj_self(XN, VSELF, wvp)

            with tc.tile_pool(name="p1b", bufs=1) as p1b, \
                 tc.tile_pool(name="tp1", bufs=1) as tp1:
                MS = p1b.tile([P, c.KK, c.SQ], F32)
                nc.sync.dma_start(
                    out=MS, in_=mself.rearrange("(k p) q -> p k q", p=P)
                )
                OSELF = p1b.tile([P, c.HH, c.SQ], F32R)

                DENS = p1b.tile([2 * c.HH, c.SQ], F32)
                DENSI = p1b.tile([2 * c.HH, c.SQ], F32)
                with tc.tile_pool(name="ps_oacc", bufs=2, space="PSUM") as psoa:
                    for hp in range(c.HH):
                        ps_o1 = psoa.tile(
                            [65, c.SQ], F32, name="ps_o1", tag="ps_o1"
                        )
                        ps_o2 = psoa.tile(
                            [65, c.SQ], F32, name="ps_o2", tag="ps_o2"
                        )
                        attention_hp(
                            hp, KHAT, VSELF, QHAT, MS, c.KK,
                            ps_o1, ps_o2, tp1, True, True,
                        )
                        evict_unnorm(ps_o1, hp, False, OSELF, DENS, tp1)
                        evict_unnorm(ps_o2, hp, True, OSELF, DENS, tp1)
                normalize_batch(OSELF, DENS, DENSI, tp1, c.HH)

                with tc.tile_pool(name="ps_gx", bufs=2, space="PSUM") as psg:
                    out_proj_residual(
                        wselfT, OSELF, lambda j: ada_gate_one(2, j, psg),
                        lambda j: stream_x(xT, j, slice(0, c.SQ)), xc_d,
                    )

        # =======================================================
        # Phase 2: cross-attention (memory quarters, LN folded into proj)
        # =======================================================
        with tc.tile_pool(name="p2", bufs=1) as p2:
            rs_b, m_b = device_ln_stats(lambda j: stream_xr(xc_d, j))
            QC = p2.tile([P, c.HH, c.SQ], F32R)
            with tc.tile_pool(name="p2q", bufs=1) as p2q:
                XNC = p2q.tile([P, c.CH, c.SQ], F32R)
                ada_modulate(
                    3, 4, lambda j, tf: stream_x(xc_d, j, slice(0, c.SQ)), 1,
                    lambda cols: rs_b[:, cols], lambda cols: m_b[:, cols],
                    lambda j, tf: XNC[:, j, :],
                )
                proj_rope(wqT, 0, c.SQ, CQ, SQt, QC, XNC)

            OACC1 = p2.tile([65, c.HH, c.SQ], F32)
            OACC2 = p2.tile([65, c.HH, c.SQ], F32)
            SWK = p2.tile([P, c.HH], F32)
            nc.sync.dma_start(out=SWK, in_=swk)
            WSVb = p2.tile([P, c.H * c.HD], F32)
            _dma_bcast(nc, WSVb, wsumv, 0, c.H * c.HD)
            RSC = p2.tile([P, 2 * c.KK], F32)
            nc.sync.dma_start(out=RSC, in_=rs_cols)
            MRSC = p2.tile([P, 2 * c.KK], F32)
            nc.sync.dma_start(out=MRSC, in_=mrs_cols)

            nq = 2 * c.NF  # memory quarters over the 2N tokens
            for qq in range(nq):
                half = qq // c.NF            # 0: clean, 1: observed
                hq = qq % c.NF               # quarter index within half
                memT = hcT if half == 0 else hoT
                la_m = la_mc if half == 0 else la_mo
                lb_m = lb_mc if half == 0 else lb_mo
                mmask = mhc if half == 0 else mho
                tok0 = hq * c.SQ             # position offset within half
                ctok = slice(tok0, tok0 + c.SQ)

                with tc.tile_pool(name="p2h", bufs=1) as p2h, \
                     tc.tile_pool(name="mstr", bufs=1) as mstr:
                    MEMQ = p2h.tile([P, c.CH, c.SQ], F32R)
                    nc.sync.dma_start(
                        out=MEMQ,
                        in_=memT[:, ctok].rearrange("(k p) n -> p k n", p=P),
                    )
                    KC = p2h.tile([P, c.HH, c.SQ], F32R)
                    VC = p2h.tile([P, c.QKK, c.H * 65], F32R)
                    CKm_t = p2h.tile([P, c.SQ], F32)
                    nc.sync.dma_start(out=CKm_t, in_=cktm[:, ctok])
                    SKm_t = p2h.tile([P, c.SQ], F32)
                    nc.sync.dma_start(out=SKm_t, in_=sktm[:, ctok])
                    LAm = p2h.tile([P, c.SQ], F32)
                    _dma_bcast(nc, LAm, la_m, tok0, c.SQ)
                    LBm = p2h.tile([P, c.SQ], F32)
                    _dma_bcast(nc, LBm, lb_m, tok0, c.SQ)

                    # ---- K projection: 8 psum banks, stream raw memory ----
                    with tc.tile_pool(name="ps_kp", bufs=1, space="PSUM") as pkp:
                        pks = [
                            pkp.tile([P, c.SQ], F32, name=f"ps_k{h}",
                                     tag=f"ps_k{h}")
                            for h in range(c.HH)
                        ]
                        for hh in range(c.HH):
                            wth = wbtile(c.CH)
                            nc.sync.dma_start(
                                out=wth,
                                in_=wkvT[:, hh * P : (hh + 1) * P].rearrange(
                                    "(k p) m -> p k m", p=P
                                ),
                            )
                            for k in range(c.CH):
                                nc.tensor.matmul(
                                    pks[hh], wth[:, k, :], MEMQ[:, k, :],
                                    start=(k == 0), stop=(k == c.CH - 1),
                                )
                        for hh in range(c.HH):
                            # LN fold: z = ps*rs_t - (mu*rs)_t * rowsum(Wk)
                            t2 = tw()
                            nc.vector.tensor_scalar_mul(
                                t2, LBm, SWK[:, hh : hh + 1]
                            )
                            t1 = tw()
                            nc.vector.tensor_mul(t1, pks[hh], LAm)
                            z = tw()
                            nc.vector.tensor_sub(z, t1, t2)
                            rope_evict(
                                z, hh, slice(0, c.SQ), CKm_t, SKm_t, KC
                            )

                    # ---- V projection (token-major quarter) ----
                    ffw = min(512, c.H * c.HD)
                    nff = (c.H * c.HD) // ffw
                    hpf = ffw // 64
                    for tt in range(c.QKK):
                        ap = VC[:, tt, :].rearrange(
                            "p (h e) -> p h e", e=65
                        )[:, :, 64:65]
                        nc.vector.tensor_copy(ap, ONES16[:, 0 : c.H])
                    with tc.tile_pool(name="ps_v2", bufs=4, space="PSUM") as psv:
                        for ff in range(nff):
                            pss = [
                                psv.tile([P, ffw], F32, name="ps_v2",
                                         tag="ps_v2")
                                for _ in range(c.QKK)
                            ]
                            kh = max(1, c.CH // 4)
                            for kg in range(c.CH // kh):
                                wt = mstr.tile([P, kh, ffw], F32R, name="wv",
                                               tag="wv", bufs=2)
                                nc.sync.dma_start(
                                    out=wt,
                                    in_=wkvT[
                                        kg * kh * P : (kg + 1) * kh * P,
                                        c.D + ff * ffw : c.D + (ff + 1) * ffw,
                                    ].rearrange("(k p) m -> p k m", p=P),
                                )
                                for k in range(kh):
                                    gk = kg * kh + k
                                    for tt in range(c.QKK):
                                        nc.tensor.matmul(
                                            pss[tt],
                                            MEMQ[:, gk, tt * P : (tt + 1) * P],
                                            wt[:, k, :],
                                            start=(gk == 0),
                                            stop=(gk == c.CH - 1),
                                        )
                            for tt in range(c.QKK):
                                tok_col = half * c.KK + hq * c.QKK + tt
                                t2 = mstr.tile(
                                    [P, ffw], F32, name="tvw", tag="tvw",
                                    bufs=2,
                                )
                                nc.vector.tensor_scalar_mul(
                                    t2, WSVb[:, ff * ffw : (ff + 1) * ffw],
                                    MRSC[:, tok_col : tok_col + 1],
                                )
                                ap = VC[
                                    :, tt, ff * hpf * 65 : (ff + 1) * hpf * 65
                                ].rearrange("p (h e) -> p h e", e=65)[:, :, 0:64]
                                nc.vector.scalar_tensor_tensor(
                                    out=ap, in0=pss[tt],
                                    scalar=RSC[:, tok_col : tok_col + 1],
                                    in1=t2, op0=OP.mult, op1=OP.subtract,
                                )

                    # ---- attention over this quarter ----
                    with tc.tile_pool(name="p2ha", bufs=1) as p2ha, \
                         tc.tile_pool(name="tp2", bufs=1) as tp2:
                        MKq = p2ha.tile([P, c.QKK, c.SQ], F32)
                        nc.sync.dma_start(
                            out=MKq,
                            in_=mmask[tok0 : tok0 + c.SQ, :].rearrange(
                                "(k p) q -> p k q", p=P
                            ),
                        )
                        with tc.tile_pool(
                            name="ps_oc", bufs=2, space="PSUM"
                        ) as psoc:
                            for hp in range(c.HH):
                                ps_o1 = psoc.tile(
                                    [65, c.SQ], F32, name="ps_oc1",
                                    tag="ps_oc1",
                                )
                                ps_o2 = psoc.tile(
                                    [65, c.SQ], F32, name="ps_oc2",
                                    tag="ps_oc2",
                                )
                                attention_hp(
                                    hp, KC, VC, QC, MKq, c.QKK,
                                    ps_o1, ps_o2, tp2, True, True,
                                )
                                if qq == 0:
                                    nc.vector.tensor_copy(
                                        OACC1[:, hp, :], ps_o1
                                    )
                                    nc.vector.tensor_copy(
                                        OACC2[:, hp, :], ps_o2
                                    )
                                else:
                                    nc.vector.tensor_add(
                                        OACC1[:, hp, :], OACC1[:, hp, :],
                                        ps_o1,
                                    )
                                    nc.vector.tensor_add(
                                        OACC2[:, hp, :], OACC2[:, hp, :],
                                        ps_o2,
                                    )

            with tc.tile_pool(name="p2n", bufs=1) as p2n:
                OC = p2n.tile([P, c.HH, c.SQ], F32R)
                DENC = p2n.tile([2 * c.HH, c.SQ], F32)
                DENCI = p2n.tile([2 * c.HH, c.SQ], F32)
                with tc.tile_pool(name="tp2n", bufs=1) as tp2n:
                    for hp in range(c.HH):
                        nc.sync.dma_start(
                            out=DENC[2 * hp : 2 * hp + 1, :],
                            in_=OACC1[64:65, hp, :],
                        )
                        nc.sync.dma_start(
                            out=DENC[2 * hp + 1 : 2 * hp + 2, :],
                            in_=OACC2[64:65, hp, :],
                        )
                    nc.vector.reciprocal(DENCI, DENC)
                    for hp in range(c.HH):
                        d1 = small.tile([1, c.SQ], F32, name="s_d1",
                                        tag="s_d1", bufs=2)
                        nc.sync.dma_start(
                            out=d1, in_=DENCI[2 * hp : 2 * hp + 1, :]
                        )
                        d2 = small.tile([1, c.SQ], F32, name="s_d2",
                                        tag="s_d2", bufs=2)
                        nc.sync.dma_start(
                            out=d2, in_=DENCI[2 * hp + 1 : 2 * hp + 2, :]
                        )
                        rb = tp2n.tile(
                            [64, c.SQ], F32, name="t_rb", tag="t_rb", bufs=2
                        )
                        nc.gpsimd.partition_broadcast(rb, d1, channels=64)
                        nc.vector.tensor_mul(
                            OC[0:64, hp, :], OACC1[0:64, hp, :], rb
                        )
                        rh = tp2n.tile(
                            [64, c.SQ], F32, name="t_rh", tag="t_rh", bufs=2
                        )
                        nc.gpsimd.partition_broadcast(rh, d2, channels=64)
                        st = tp2n.tile(
                            [64, c.SQ], F32R, name="t_onorm", tag="t_onorm",
                            bufs=2,
                        )
                        nc.vector.tensor_mul(st, OACC2[0:64, hp, :], rh)
                        nc.sync.dma_start(out=OC[64:128, hp, :], in_=st)
                with tc.tile_pool(name="ps_gx", bufs=2, space="PSUM") as psg:
                    out_proj_residual(
                        wcrossT, OC, lambda j: ada_gate_one(5, j, psg),
                        lambda j: stream_x(xc_d, j, slice(0, c.SQ)), xc2_d,
                    )

        # =======================================================
        # Phase 3: MLP (two hidden halves, SBUF accumulation)
        # =======================================================
        with tc.tile_pool(name="p3", bufs=1) as p3:
            rs_b, m_b = device_ln_stats(lambda j: stream_xr(xc2_d, j))
            OUT_ACC = p3.tile([P, c.CH, c.SQ], F32)

            with tc.tile_pool(name="p3x", bufs=1) as p3x:
                XNM = p3x.tile([P, c.CH, c.SQ], F32R)
                ada_modulate(
                    6, 7, lambda j, tf: stream_x(xc2_d, j, slice(0, c.SQ)), 1,
                    lambda cols: rs_b[:, cols], lambda cols: m_b[:, cols],
                    lambda j, tf: XNM[:, j, :],
                )
                nhalf = c.DHC // 2
                for half in range(2):
                    with tc.tile_pool(name="p3h", bufs=1) as p3h:
                        HT = p3h.tile([P, nhalf, c.SQ], F32R)
                        with tc.tile_pool(
                            name="ps_m1", bufs=3, space="PSUM"
                        ) as psm:
                            for jj in range(nhalf):
                                gj = half * nhalf + jj
                                ps = psm.tile(
                                    [P, c.SQ], F32, name="ps_m1", tag="ps_m1"
                                )
                                wt = wbtile(c.CH)
                                nc.sync.dma_start(
                                    out=wt,
                                    in_=wm1T[
                                        :, gj * P : (gj + 1) * P
                                    ].rearrange("(k p) m -> p k m", p=P),
                                )
                                for k in range(c.CH):
                                    nc.tensor.matmul(
                                        ps, wt[:, k, :], XNM[:, k, :],
                                        start=(k == 0), stop=(k == c.CH - 1),
                                    )
                                nc.scalar.activation(
                                    HT[:, jj, :], ps, AF.Gelu_apprx_tanh,
                                    bias=BM1[:, gj : gj + 1],
                                )
                        with tc.tile_pool(
                            name="ps_m2", bufs=3, space="PSUM"
                        ) as psm2:
                            for j in range(c.CH):
                                ps = psm2.tile(
                                    [P, c.SQ], F32, name="ps_m2", tag="ps_m2"
                                )
                                wt = p3h.tile(
                                    [P, nhalf, P], F32R, name="wm2b",
                                    tag="wm2b", bufs=2,
                                )
                                nc.sync.dma_start(
                                    out=wt,
                                    in_=wm2T[
                                        half * nhalf * P : (half + 1) * nhalf * P,
                                        j * P : (j + 1) * P,
                                    ].rearrange("(k p) m -> p k m", p=P),
                                )
                                for kk_ in range(nhalf):
                                    nc.tensor.matmul(
                                        ps, wt[:, kk_, :], HT[:, kk_, :],
                                        start=(kk_ == 0),
                                        stop=(kk_ == nhalf - 1),
                                    )
                                if half == 0:
                                    nc.vector.tensor_copy(OUT_ACC[:, j, :], ps)
                                else:
                                    nc.vector.tensor_add(
                                        OUT_ACC[:, j, :], OUT_ACC[:, j, :], ps
                                    )

            with tc.tile_pool(name="p3o", bufs=1) as p3o, \
                 tc.tile_pool(name="ps_gx", bufs=2, space="PSUM") as psg:
                OUT = p3o.tile([P, c.CH, c.SQ], F32)
                for j in range(c.CH):
                    gj = ada_gate_one(8, j, psg)
                    t = tw()
                    nc.vector.scalar_tensor_tensor(
                        out=t, in0=OUT_ACC[:, j, :], scalar=BM2[:, j : j + 1],
                        in1=gj, op0=OP.add, op1=OP.mult,
                    )
                    xrj = stream_x(xc2_d, j, slice(0, c.SQ))
                    nc.vector.tensor_add(OUT[:, j, :], t, xrj)
                nc.sync.dma_start(
                    out=out_d.rearrange("(k p) q -> p k q", p=P), in_=OUT
                )

    nc.compile()
    return nc


# =======================================================
# Host side
# =======================================================

def host_prep(cfg: Cfg, inputs: dict):
    c = cfg
    f32 = np.float32

    q_x = np.asarray(inputs["q_x"], f32)
    h_content = np.asarray(inputs["h_content"], f32)
    h_obs = np.asarray(inputs["h_obs"], f32)
    t_cond = np.asarray(inputs["t_cond"], f32)
    M_QQ = np.asarray(inputs["M_QQ"], f32)
    M_hyb = np.asarray(inputs["M_hyb"], f32)
    w_ln_self = np.asarray(inputs["w_ln_self"], f32)
    w_qkv = np.asarray(inputs["w_qkv"], f32)
    w_self_out = np.asarray(inputs["w_self_out"], f32)
    w_ln_cross = np.asarray(inputs["w_ln_cross"], f32)
    w_ln_mem = np.asarray(inputs["w_ln_mem"], f32)
    w_qproj = np.asarray(inputs["w_qproj"], f32)
    w_kvproj = np.asarray(inputs["w_kvproj"], f32)
    w_cross_out = np.asarray(inputs["w_cross_out"], f32)
    w_ln_mlp = np.asarray(inputs["w_ln_mlp"], f32)
    w_mlp1 = np.asarray(inputs["w_mlp1"], f32)
    b_mlp1 = np.asarray(inputs["b_mlp1"], f32)
    w_mlp2 = np.asarray(inputs["w_mlp2"], f32)
    b_mlp2 = np.asarray(inputs["b_mlp2"], f32)
    w_ada = np.asarray(inputs["w_ada"], f32)
    b_ada = np.asarray(inputs["b_ada"], f32)

    D, N, HD, SQ = c.D, c.N, c.HD, c.SQ

    wada9 = w_ada[: 9 * D].copy()
    bada9 = b_ada[: 9 * D].copy()
    for q, wl in ((1, w_ln_self), (4, w_ln_cross), (7, w_ln_mlp)):
        wada9[q * D : (q + 1) * D] *= wl[:, None]
        bada9[q * D : (q + 1) * D] = wl * (1.0 + b_ada[q * D : (q + 1) * D])
    wadaT = np.ascontiguousarray(wada9.T)
    bada_h = np.ascontiguousarray(bada9.reshape(9 * c.CH, P).T)

    wqkvT = np.ascontiguousarray(w_qkv.T)
    wselfT = np.ascontiguousarray(w_self_out.T)
    wqT = np.ascontiguousarray(w_qproj.T)
    wkv_eff = w_kvproj * w_ln_mem[None, :]
    wkvT = np.ascontiguousarray(wkv_eff.T)
    wcrossT = np.ascontiguousarray(w_cross_out.T)
    wm1T = np.ascontiguousarray(w_mlp1.T)
    wm2T = np.ascontiguousarray(w_mlp2.T)
    bm1_h = np.ascontiguousarray(b_mlp1.reshape(c.DHC, P).T)
    bm2_h = np.ascontiguousarray(b_mlp2.reshape(c.CH, P).T)

    # rowsums for the folded memory layernorm
    wsum = wkv_eff.sum(1).astype(f32)          # [2D]
    swk_h = np.ascontiguousarray(wsum[:D].reshape(c.HH, P).T)
    wsumv_h = np.ascontiguousarray(wsum[D:][None, :])

    pos = np.arange(N, dtype=f32)
    inv = (10000.0 ** (-np.arange(0, HD, 2, dtype=f32) / HD)).astype(f32)
    freqs = pos[:, None] * inv[None, :]
    cos64 = np.concatenate([np.cos(freqs), np.cos(freqs)], 1)
    s_sgn = np.concatenate([-np.sin(freqs), np.sin(freqs)], 1)
    c_pair = np.ascontiguousarray(np.tile(cos64.T, (2, 1)).astype(f32))
    s_pair = np.ascontiguousarray(np.tile(s_sgn.T, (2, 1)).astype(f32))
    scale = f32(1.0 / np.sqrt(HD))

    in_maps = []
    for b in range(c.B):
        xb = q_x[b]
        mu_x = xb.mean(-1).astype(f32)
        rs_x = (1.0 / np.sqrt(xb.var(-1) + c.eps)).astype(f32)
        mem = np.concatenate([h_content[b], h_obs[b]], 0)
        mu_m = mem.mean(-1).astype(f32)
        rs_m = (1.0 / np.sqrt(mem.var(-1) + c.eps)).astype(f32)
        mrs_m = (mu_m * rs_m).astype(f32)
        rs_cols_h = np.ascontiguousarray(rs_m.reshape(2 * c.KK, P).T)
        mrs_cols_h = np.ascontiguousarray(mrs_m.reshape(2 * c.KK, P).T)
        mTQQ = np.exp(np.minimum(M_QQ[b].T, 0.0)).astype(f32)
        mThyb = np.exp(np.minimum(M_hyb[b].T, 0.0)).astype(f32)

        for s in range(2):
            own = np.arange(s * SQ, (s + 1) * SQ)
            rest = np.concatenate(
                [np.arange(0, s * SQ), np.arange((s + 1) * SQ, N)]
            )
            perm = np.concatenate([own, rest]).astype(np.int64)
            im = {
                "xT": np.ascontiguousarray(xb.T[:, perm]),
                "tcT": np.ascontiguousarray(t_cond[b].T[:, perm]),
                "hcT": np.ascontiguousarray(h_content[b].T),
                "hoT": np.ascontiguousarray(h_obs[b].T),
                "wadaT": wadaT, "wqkvT": wqkvT, "wselfT": wselfT,
                "wqT": wqT, "wkvT": wkvT, "wcrossT": wcrossT,
                "wm1T": wm1T, "wm2T": wm2T,
                "bada": bada_h, "bm1": bm1_h, "bm2": bm2_h,
                "cqt": np.ascontiguousarray(c_pair[:, perm[:SQ]] * scale),
                "sqt": np.ascontiguousarray(s_pair[:, perm[:SQ]] * scale),
                "ckts": np.ascontiguousarray(c_pair[:, perm]),
                "skts": np.ascontiguousarray(s_pair[:, perm]),
                "cktm": c_pair, "sktm": s_pair,
                "mself": np.ascontiguousarray(mTQQ[perm][:, perm[:SQ]]),
                "mhc": np.ascontiguousarray(mThyb[:N][:, perm[:SQ]]),
                "mho": np.ascontiguousarray(mThyb[N:][:, perm[:SQ]]),
                "la_self": np.ascontiguousarray(rs_x[perm][None, :]),
                "lb_self": np.ascontiguousarray(mu_x[perm][None, :]),
                "la_mc": np.ascontiguousarray(rs_m[:N][None, :]),
                "lb_mc": np.ascontiguousarray(mrs_m[:N][None, :]),
                "la_mo": np.ascontiguousarray(rs_m[N:][None, :]),
                "lb_mo": np.ascontiguousarray(mrs_m[N:][None, :]),
                "swk": swk_h, "wsumv": wsumv_h,
                "rs_cols": rs_cols_h, "mrs_cols": mrs_cols_h,
            }
            in_maps.append(im)
    return in_maps


_PROGRAM_CACHE = {}


def get_program(cfg: Cfg):
    key = (cfg.N, cfg.D, cfg.H)
    if key not in _PROGRAM_CACHE:
        _PROGRAM_CACHE[key] = build_program(cfg)
    return _PROGRAM_CACHE[key]


def assemble(cfg: Cfg, results):
    c = cfg
    out = np.zeros((c.B, c.N, c.D), np.float32)
    for b in range(c.B):
        for s in range(2):
            o = results[2 * b + s]["out"]
            out[b, s * c.SQ : (s + 1) * c.SQ, :] = o.T
    return out


def kernel(**inputs) -> np.ndarray:
    cfg = Cfg(mini=False)
    nc = get_program(cfg)
    in_maps = host_prep(cfg, inputs)
    res = bass_utils.run_bass_kernel_spmd(
        nc, in_maps, core_ids=list(range(cfg.n_cores)), trace=False
    )
    return assemble(cfg, res.results)



# revision 16
# speedup vs baseline: 1.6564x; 1.6564x over previous
"""Trainium2 Bass kernel for a DiT-style transformer block (adaLN modulation,
RoPE self-attention with additive rank mask, hybrid cross-attention to
[clean|observed] memory, gated MLP).

Sharding: 8 cores = 4 batches x 2 sequence-halves. Each core computes the
block output for its 512 query tokens of one batch. Per-core token order is
permuted (host side) so the core's own tokens come first.

v2 design notes:
- All matmul operands are bf16 (PE full rate, FWL weight loads, half DMA,
  2x DVE on elementwise ops). PSUM accumulation stays fp32; LN statistics,
  softmax denominators and the residual stream stay fp32.
- Everything that depends only on kernel inputs is precomputed on the host:
  the 9 used adaLN fields (t_cond @ w_ada.T + b_ada), the fully modulated
  self-attention input xn_self, the layernormed memory, exp(mask) in {0,1},
  and scaled RoPE tables.
- Scores for a head pair run as two concurrent K=64 matmuls in disjoint PE
  row groups (partitions 0:64 / 64:128). p@v uses the ones-column trick for
  softmax denominators (v tile has 65 columns; row 64 of o is the denom).
- Activations stay resident in SBUF between phases (no DRAM roundtrip).
- The RoPE rotate-half partition shift is done with 4 batched SBUF-SBUF
  DMAs per projection over all 8 head-pairs at once.
"""

import numpy as np
import ml_dtypes
from contextlib import ExitStack

from concourse import bacc, mybir
import concourse.bass as bass
import concourse.tile as tile
from concourse import bass_utils

F32 = mybir.dt.float32
F32R = mybir.dt.float32r
BF16 = mybir.dt.bfloat16
AF = mybir.ActivationFunctionType
OP = mybir.AluOpType

P = 128
BF = ml_dtypes.bfloat16


class Cfg:
    def __init__(self, mini=False):
        self.B, self.N, self.D, self.H, self.HD = 4, 1024, 1024, 16, 64
        self.COND = 256
        self.DH = 4 * self.D
        self.SQ = self.N // 2            # own query tokens per core
        self.CH = self.D // P            # feature chunks (8)
        self.HH = self.H * self.HD // P  # head-pair chunks (8)
        self.KK = self.N // P            # self key chunks (8)
        self.MKK = 2 * self.N // P       # memory key chunks (16)
        self.DHC = self.DH // P          # mlp hidden chunks (32)
        self.n_cores = 2 * self.B
        self.eps = 1e-5


def build_program(cfg: Cfg):
    c = cfg
    nc = bacc.Bacc(
        "TRN2",
        target_bir_lowering=False,
        debug=False,
        enable_asserts=True,
        num_devices=c.n_cores,
    )

    def din(name, shape, dt=BF16):
        return nc.dram_tensor(name, shape, dt, kind="ExternalInput").ap()

    xnT = din("xnT", [c.D, c.N])            # modulated ln(q_x), feature-major
    xrT = din("xrT", [c.D, c.SQ], F32)      # residual stream (own tokens)
    hnT = din("hnT", [c.D, 2 * c.N])        # normalized memory [clean|obs]
    wqkvT = din("wqkvT", [c.D, 3 * c.D])
    wselfT = din("wselfT", [c.D, c.D])
    wqT = din("wqT", [c.D, c.D])
    wkvT = din("wkvT", [c.D, 2 * c.D])
    wcrossT = din("wcrossT", [c.D, c.D])
    wm1T = din("wm1T", [c.D, c.DH])
    wm2T = din("wm2T", [c.DH, c.D])
    bm1 = din("bm1", [P, c.DHC], F32)
    bm2 = din("bm2", [P, c.CH], F32)
    gs_f = din("gs", [c.D, c.SQ])           # adaLN fields (host-computed)
    shc_f = din("shc", [c.D, c.SQ])
    scc_f = din("scc", [c.D, c.SQ])         # = w_ln_cross*(1+sc_c)
    gc_f = din("gc", [c.D, c.SQ])
    shm_f = din("shm", [c.D, c.SQ])
    scm_f = din("scm", [c.D, c.SQ])
    gm_f = din("gm", [c.D, c.SQ])
    cq_t = din("cq", [P, c.SQ])             # rope tables (scale folded on Q)
    sq_t = din("sq", [P, c.SQ])
    ckS_t = din("ckS", [P, c.N])            # self keys (permuted positions)
    skS_t = din("skS", [P, c.N])
    ckM_t = din("ckM", [P, c.N])            # memory keys (natural positions)
    skM_t = din("skM", [P, c.N])
    mS_d = din("mS", [c.N, 2 * c.SQ])       # exp(mask) in {0,1}, 2-head dup
    mC_d = din("mC", [c.N, 2 * c.SQ])
    mO_d = din("mO", [c.N, 2 * c.SQ])
    out_d = nc.dram_tensor("out", [c.D, c.SQ], F32, kind="ExternalOutput").ap()

    with ExitStack() as ctx:
        tc = ctx.enter_context(tile.TileContext(nc))
        persist = ctx.enter_context(tc.tile_pool(name="persist", bufs=1))
        resid = ctx.enter_context(tc.tile_pool(name="resid", bufs=1))
        ws = ctx.enter_context(tc.tile_pool(name="ws", bufs=1))
        twbp = ctx.enter_context(tc.tile_pool(name="twb", bufs=5))
        twfp = ctx.enter_context(tc.tile_pool(name="twf", bufs=2))
        small = ctx.enter_context(tc.tile_pool(name="small", bufs=1))

        def r(ap):
            return ap.bitcast(F32)

        def twb():
            return twbp.tile([P, c.SQ], BF16, name="twb", tag="twb")

        def twf():
            return twfp.tile([P, c.SQ], F32, name="twf", tag="twf")

        def wk_tile():
            return ws.tile([P, c.CH, P], BF16, name="wk", tag="wk", bufs=4)

        def wv_tile():
            return ws.tile([P, 4, 512], BF16, name="wv", tag="wv", bufs=2)

        def wf_tile():
            return ws.tile([P, c.SQ], BF16, name="wf", tag="wf", bufs=4)

        # ---------- persistent preloads ----------
        CQ = persist.tile([P, c.SQ], BF16)
        nc.sync.dma_start(out=CQ, in_=cq_t)
        SQt = persist.tile([P, c.SQ], BF16)
        nc.sync.dma_start(out=SQt, in_=sq_t)
        BM1 = persist.tile([P, c.DHC], F32)
        nc.sync.dma_start(out=BM1, in_=bm1)
        BM2 = persist.tile([P, c.CH], F32)
        nc.sync.dma_start(out=BM2, in_=bm2)

        EPS = persist.tile([P, 1], F32)
        nc.vector.memset(EPS, 1e-5)
        ONESB = persist.tile([P, 16], BF16)
        nc.vector.memset(ONESB, 1.0)
        ones_f32 = persist.tile([P, 1], F32)
        nc.vector.memset(ones_f32, 1.0)
        ONEr = persist.tile([P, 1], F32R)
        nc.vector.tensor_copy(ONEr, ones_f32)

        XC = resid.tile([P, c.CH, c.SQ], F32R)   # residual after self-attn
        XC2 = resid.tile([P, c.CH, c.SQ], F32R)  # residual after cross-attn
        XNC = resid.tile([P, c.CH, c.SQ], BF16)  # modulated cross input
        RSB = [
            resid.tile([P, c.SQ], BF16, name=f"RSB{i}", tag=f"RSB{i}")
            for i in range(2)
        ]
        MB = [
            resid.tile([P, c.SQ], BF16, name=f"MB{i}", tag=f"MB{i}")
            for i in range(2)
        ]

        # ---------- helpers ----------
        def shift32(dst, src):
            """dst[p] = src[p xor-32 within each 64-block]."""
            for b in (0, 64):
                nc.sync.dma_start(out=dst[b : b + 32, :],
                                  in_=src[b + 32 : b + 64, :])
                nc.sync.dma_start(out=dst[b + 32 : b + 64, :],
                                  in_=src[b : b + 32, :])

        def qk_proj_rope(tag, wT, col_off, src, src_off, nf, ctab, stab,
                         dst, dst_off):
            """dst[:, hh, dst_off + t] = rope(W[:, cols].T @ src[:, :, t])."""
            nq = nf * c.SQ
            with tc.tile_pool(name=f"z_{tag}", bufs=1) as zpool:
                Z = zpool.tile([P, c.HH, nq], BF16, name="z", tag="z")
                ZS = zpool.tile([P, c.HH, nq], BF16, name="zs", tag="zs")
                with tc.tile_pool(name=f"ps_{tag}", bufs=4,
                                  space="PSUM") as psq:
                    for hh in range(c.HH):
                        wt = wk_tile()
                        nc.sync.dma_start(
                            out=wt,
                            in_=wT[
                                :, col_off + hh * P : col_off + (hh + 1) * P
                            ].rearrange("(k p) m -> p k m", p=P),
                        )
                        for tf in range(nf):
                            ps = psq.tile([P, c.SQ], F32, name="q",
                                          tag="q")
                            for k in range(c.CH):
                                nc.tensor.matmul(
                                    ps, wt[:, k, :],
                                    src[:, k,
                                        src_off + tf * c.SQ :
                                        src_off + (tf + 1) * c.SQ],
                                    start=(k == 0), stop=(k == c.CH - 1),
                                )
                            nc.scalar.activation(
                                Z[:, hh, tf * c.SQ : (tf + 1) * c.SQ], ps,
                                AF.Copy,
                            )
                shift32(ZS, Z)
                for hh in range(c.HH):
                    for tf in range(nf):
                        cs = slice(tf * c.SQ, (tf + 1) * c.SQ)
                        ds = slice(dst_off + tf * c.SQ,
                                   dst_off + (tf + 1) * c.SQ)
                        t1 = twb()
                        nc.vector.tensor_mul(t1, Z[:, hh, cs], ctab[:, cs])
                        t2 = twb()
                        nc.vector.tensor_mul(t2, ZS[:, hh, cs], stab[:, cs])
                        nc.vector.tensor_add(dst[:, hh, ds], t1, t2)

        def v_proj(tag, wT, col_off, src, tt0, ntt, vdst):
            """Token-major value projection with ones column per head."""
            for tt in range(ntt):
                ap = vdst[:, tt0 + tt, :].rearrange(
                    "p (h e) -> p h e", e=65
                )[:, :, 64:65]
                nc.vector.tensor_copy(ap, ONESB[:, 0 : c.H])
            ffw = 512
            nff = (c.H * c.HD) // ffw
            hpf = ffw // 64
            with tc.tile_pool(name=f"ps_{tag}", bufs=8, space="PSUM") as psv:
                for ff in range(nff):
                    pss = [
                        psv.tile([P, ffw], F32, name="v", tag="v")
                        for _ in range(ntt)
                    ]
                    for kg in range(2):
                        wt = wv_tile()
                        nc.sync.dma_start(
                            out=wt,
                            in_=wT[
                                kg * 4 * P : (kg + 1) * 4 * P,
                                col_off + ff * ffw : col_off + (ff + 1) * ffw,
                            ].rearrange("(k p) m -> p k m", p=P),
                        )
                        for k in range(4):
                            gk = kg * 4 + k
                            for tt in range(ntt):
                                nc.tensor.matmul(
                                    pss[tt],
                                    src[:, gk, tt * P : (tt + 1) * P],
                                    wt[:, k, :],
                                    start=(gk == 0), stop=(gk == c.CH - 1),
                                )
                    for tt in range(ntt):
                        ap = vdst[
                            :, tt0 + tt, ff * hpf * 65 : (ff + 1) * hpf * 65
                        ].rearrange("p (h e) -> p h e", e=65)[:, :, 0:64]
                        nc.vector.tensor_copy(ap, pss[tt])

        def mask_fetch(dram_rows, kk):
            """Stream one [P, 2*SQ] mask chunk (rows kk*P..) from DRAM."""
            mt = ws.tile([P, 2 * c.SQ], BF16, name="t_mk", tag="t_mk",
                         bufs=4)
            nc.sync.dma_start(out=mt, in_=dram_rows[kk * P : (kk + 1) * P, :])
            return mt

        def attention(khat, qhat, vtile, masks_fn, n_kk, OST, DEN, ptp):
            """All head pairs; unnormalized output + denominators."""
            with tc.tile_pool(name="ps_oacc", bufs=1, space="PSUM") as opso:
                for gp in range(c.HH // 2):
                    hps = (2 * gp, 2 * gp + 1)
                    ot = {}
                    for i, hp in enumerate(hps):
                        ot[hp] = (
                            opso.tile([65, c.SQ], F32, name=f"o1_{i}",
                                      tag=f"o1_{i}"),
                            opso.tile([65, c.SQ], F32, name=f"o2_{i}",
                                      tag=f"o2_{i}"),
                        )

                    def pv(hp, kk, pt):
                        o1, o2 = ot[hp]
                        h1, h2 = 2 * hp, 2 * hp + 1
                        nc.tensor.matmul(
                            o1, vtile[:, kk, h1 * 65 : (h1 + 1) * 65],
                            pt[:, 0 : c.SQ],
                            start=(kk == 0), stop=(kk == n_kk - 1),
                        )
                        nc.tensor.matmul(
                            o2, vtile[:, kk, h2 * 65 : (h2 + 1) * 65],
                            pt[:, c.SQ : 2 * c.SQ],
                            start=(kk == 0), stop=(kk == n_kk - 1),
                        )

                    pending = []
                    with tc.tile_pool(name="ps_s", bufs=2,
                                      space="PSUM") as pss:
                        for kk in range(n_kk):
                            mt = masks_fn(kk)
                            for hp in hps:
                                ps = pss.tile([P, 2 * c.SQ], F32,
                                              name="ps_s", tag="ps_s")
                                ks = slice(kk * P, (kk + 1) * P)
                                nc.tensor.matmul(
                                    ps[:, 0 : c.SQ],
                                    khat[0:64, hp, ks], qhat[0:64, hp, :],
                                    start=True, stop=True,
                                )
                                nc.tensor.matmul(
                                    ps[:, c.SQ : 2 * c.SQ],
                                    khat[64:128, hp, ks],
                                    qhat[64:128, hp, :],
                                    start=True, stop=True,
                                )
                                pt = ptp.tile([P, 2 * c.SQ], BF16,
                                              name="t_p", tag="t_p", bufs=4)
                                nc.scalar.activation(pt, ps, AF.Exp)
                                eng = nc.vector if ((kk + hp) % 2 == 0) \
                                    else nc.gpsimd
                                eng.tensor_mul(pt, pt, mt)
                                if len(pending) >= 2:
                                    pv(*pending.pop(0))
                                pending.append((hp, kk, pt))
                        for e in pending:
                            pv(*e)
                    for hp in hps:
                        o1, o2 = ot[hp]
                        st = twf()
                        nc.vector.tensor_copy(st[64:65, :], o1[64:65, :])
                        nc.sync.dma_start(out=DEN[2 * hp : 2 * hp + 1, :],
                                          in_=st[64:65, :])
                        st2 = twf()
                        nc.vector.tensor_copy(st2[64:65, :], o2[64:65, :])
                        nc.sync.dma_start(
                            out=DEN[2 * hp + 1 : 2 * hp + 2, :],
                            in_=st2[64:65, :],
                        )
                        nc.vector.tensor_copy(OST[0:64, hp, :], o1[0:64, :])
                        sthi = twb()
                        nc.vector.tensor_copy(sthi[0:64, :], o2[0:64, :])
                        nc.sync.dma_start(out=OST[64:128, hp, :],
                                          in_=sthi[0:64, :])

        def normalize(OST, DEN, DENI, DENIB, bpool):
            nc.vector.reciprocal(DENI, DEN)
            nc.vector.tensor_copy(DENIB, DENI)
            for hp in range(c.HH):
                d1 = small.tile([1, c.SQ], BF16, name="s_d1", tag="s_d1",
                                bufs=2)
                nc.sync.dma_start(out=d1, in_=DENIB[2 * hp : 2 * hp + 1, :])
                d2 = small.tile([1, c.SQ], BF16, name="s_d2", tag="s_d2",
                                bufs=2)
                nc.sync.dma_start(out=d2,
                                  in_=DENIB[2 * hp + 1 : 2 * hp + 2, :])
                rb = bpool.tile([P, c.SQ], BF16, name="t_rb", tag="t_rb",
                                bufs=2)
                nc.gpsimd.partition_broadcast(rb[0:64, :], d1, channels=64)
                rh = bpool.tile([64, c.SQ], BF16, name="t_rh", tag="t_rh",
                                bufs=2)
                nc.gpsimd.partition_broadcast(rh, d2, channels=64)
                nc.sync.dma_start(out=rb[64:128, :], in_=rh)
                nc.vector.tensor_mul(OST[:, hp, :], OST[:, hp, :], rb)

        def out_proj(tag, wT, osrc, g_dram, xres, xdst, st1, st2, sqpool):
            """xdst[:,j,:] = xres(j) + g_j * (W.T @ o); accumulates LN
            stats of xdst into st1/st2 (PSUM [1, SQ])."""
            with tc.tile_pool(name=f"ps_{tag}", bufs=3, space="PSUM") as pso:
                for j in range(c.CH):
                    ps = pso.tile([P, c.SQ], F32, name="op", tag="op")
                    wt = wk_tile()
                    nc.sync.dma_start(
                        out=wt,
                        in_=wT[:, j * P : (j + 1) * P].rearrange(
                            "(k p) m -> p k m", p=P
                        ),
                    )
                    for hp in range(c.HH):
                        nc.tensor.matmul(
                            ps, wt[:, hp, :], osrc[:, hp, :],
                            start=(hp == 0), stop=(hp == c.HH - 1),
                        )
                    gt = wf_tile()
                    nc.sync.dma_start(out=gt,
                                      in_=g_dram[j * P : (j + 1) * P, :])
                    t = twb()
                    nc.vector.tensor_mul(t, ps, gt)
                    nc.vector.tensor_add(xdst[:, j, :], t, xres(j))
                    sq = sqpool.tile([P, c.SQ], F32R, name="sq", tag="sq",
                                     bufs=2)
                    nc.scalar.activation(sq, r(xdst[:, j, :]), AF.Square)
                    nc.tensor.matmul(
                        st1, ONEr, xdst[:, j, :],
                        start=(j == 0), stop=(j == c.CH - 1),
                    )
                    nc.tensor.matmul(
                        st2, ONEr, sq,
                        start=(j == 0), stop=(j == c.CH - 1),
                    )

        def stats_finish(st1, st2, rs_b, m_b):
            """st1/st2 PSUM [1, SQ] -> broadcast (rstd, mean) bf16 tiles."""
            m = small.tile([1, c.SQ], F32, name="s_a", tag="s_a", bufs=2)
            nc.vector.tensor_scalar_mul(m, st1[0:1, :], 1.0 / c.D)
            e2 = small.tile([1, c.SQ], F32, name="s_b", tag="s_b", bufs=1)
            nc.vector.tensor_scalar_mul(e2, st2[0:1, :], 1.0 / c.D)
            msq = small.tile([1, c.SQ], F32, name="s_c", tag="s_c", bufs=1)
            nc.vector.tensor_mul(msq, m, m)
            var = small.tile([1, c.SQ], F32, name="s_a", tag="s_a", bufs=2)
            nc.vector.tensor_sub(var, e2, msq)
            sd = small.tile([1, c.SQ], F32, name="s_b", tag="s_b", bufs=1)
            nc.scalar.activation(sd, var, AF.Sqrt, bias=EPS[0:1, :])
            rs = small.tile([1, c.SQ], F32, name="s_c", tag="s_c", bufs=1)
            nc.vector.reciprocal(rs, sd)
            rsb = small.tile([1, c.SQ], BF16, name="s_rb", tag="s_rb",
                             bufs=2)
            nc.vector.tensor_copy(rsb, rs)
            mb = small.tile([1, c.SQ], BF16, name="s_mb", tag="s_mb",
                            bufs=2)
            nc.vector.tensor_copy(mb, m)
            nc.gpsimd.partition_broadcast(rs_b, rsb, channels=P)
            nc.gpsimd.partition_broadcast(m_b, mb, channels=P)

        def modulate(xsrc, rs_b, m_b, sh_dram, sc_dram, dst):
            """dst[:,j,:] = (xsrc_j - m)*rs*sc_j + sh_j  (bf16 out)."""
            for j in range(c.CH):
                sct = wf_tile()
                nc.sync.dma_start(out=sct,
                                  in_=sc_dram[j * P : (j + 1) * P, :])
                sht = wf_tile()
                nc.sync.dma_start(out=sht,
                                  in_=sh_dram[j * P : (j + 1) * P, :])
                A = twb()
                nc.vector.tensor_mul(A, rs_b, sct)
                u = twb()
                nc.gpsimd.tensor_sub(u, r(xsrc[:, j, :]), m_b)
                v = twb()
                nc.vector.tensor_mul(v, u, A)
                nc.vector.tensor_add(dst[:, j, :], v, sht)

        # =======================================================
        # Phase 1: self-attention
        # =======================================================
        with tc.tile_pool(name="p1", bufs=1) as p1:
            QHAT = p1.tile([P, c.HH, c.SQ], BF16)
            KHAT = p1.tile([P, c.HH, c.N], BF16)
            VSELF = p1.tile([P, c.KK, c.H * 65], BF16)

            with tc.tile_pool(name="p1a", bufs=1) as p1a:
                XN = p1a.tile([P, c.CH, c.N], BF16)
                nc.sync.dma_start(
                    out=XN, in_=xnT.rearrange("(k p) n -> p k n", p=P)
                )
                CKS = p1a.tile([P, c.N], BF16)
                nc.sync.dma_start(out=CKS, in_=ckS_t)
                SKS = p1a.tile([P, c.N], BF16)
                nc.sync.dma_start(out=SKS, in_=skS_t)
                qk_proj_rope("q1", wqkvT, 0, XN, 0, 1, CQ, SQt, QHAT, 0)
                qk_proj_rope("k1", wqkvT, c.D, XN, 0, 2, CKS, SKS, KHAT, 0)
                v_proj("v1", wqkvT, 2 * c.D, XN, 0, c.KK, VSELF)

            with tc.tile_pool(name="p1o", bufs=1) as p1o:
                OSELF = p1o.tile([P, c.HH, c.SQ], BF16)
                DENS = p1o.tile([2 * c.HH, c.SQ], F32)
                DENSI = p1o.tile([2 * c.HH, c.SQ], F32)
                DENSIB = p1o.tile([2 * c.HH, c.SQ], BF16)
                with tc.tile_pool(name="p1b", bufs=1) as p1b:
                    attention(KHAT, QHAT, VSELF,
                              lambda kk: mask_fetch(mS_d, kk),
                              c.KK, OSELF, DENS, p1b)
                    normalize(OSELF, DENS, DENSI, DENSIB, p1b)

                with tc.tile_pool(name="p1c", bufs=1) as p1c, \
                     tc.tile_pool(name="ps_st1", bufs=1,
                                  space="PSUM") as psst:
                    st1 = psst.tile([1, c.SQ], F32, name="st1", tag="st1")
                    st2 = psst.tile([1, c.SQ], F32, name="st2", tag="st2")

                    def xres1(j):
                        t = twf()
                        nc.sync.dma_start(out=t,
                                          in_=xrT[j * P : (j + 1) * P, :])
                        return t

                    out_proj("op1", wselfT, OSELF, gs_f, xres1, XC,
                             st1, st2, p1c)
                    stats_finish(st1, st2, RSB[0], MB[0])
                    modulate(XC, RSB[0], MB[0], shc_f, scc_f, XNC)

        # =======================================================
        # Phase 2: cross-attention
        # =======================================================
        with tc.tile_pool(name="p2", bufs=1) as p2:
            QC = p2.tile([P, c.HH, c.SQ], BF16)
            KC = p2.tile([P, c.HH, 2 * c.N], BF16)
            VC = p2.tile([P, c.MKK, c.H * 65], BF16)
            CKMt = p2.tile([P, c.N], BF16)
            nc.sync.dma_start(out=CKMt, in_=ckM_t)
            SKMt = p2.tile([P, c.N], BF16)
            nc.sync.dma_start(out=SKMt, in_=skM_t)

            qk_proj_rope("q2", wqT, 0, XNC, 0, 1, CQ, SQt, QC, 0)

            # K/V projection over the 2048 memory tokens, quarter by quarter
            for q in range(4):
                pos0 = (q % 2) * c.SQ
                with tc.tile_pool(name="p2kv", bufs=1) as p2kv:
                    HNQ = p2kv.tile([P, c.CH, c.SQ], BF16)
                    nc.sync.dma_start(
                        out=HNQ,
                        in_=hnT[:, q * c.SQ : (q + 1) * c.SQ].rearrange(
                            "(k p) n -> p k n", p=P
                        ),
                    )
                    qk_proj_rope("k2", wkvT, 0, HNQ, 0, 1,
                                 CKMt[:, pos0 : pos0 + c.SQ],
                                 SKMt[:, pos0 : pos0 + c.SQ],
                                 KC, q * c.SQ)
                    v_proj("v2", wkvT, c.D, HNQ, q * 4, 4, VC)

            with tc.tile_pool(name="p2b", bufs=1) as p2b:
                OC = p2b.tile([P, c.HH, c.SQ], BF16)
                DENC = p2b.tile([2 * c.HH, c.SQ], F32)
                DENCI = p2b.tile([2 * c.HH, c.SQ], F32)
                DENCIB = p2b.tile([2 * c.HH, c.SQ], BF16)

                def cross_mask(kk):
                    if kk < c.KK:
                        return mask_fetch(mC_d, kk)
                    return mask_fetch(mO_d, kk - c.KK)

                attention(KC, QC, VC, cross_mask, c.MKK, OC, DENC, p2b)
                normalize(OC, DENC, DENCI, DENCIB, p2b)

                with tc.tile_pool(name="p2c", bufs=1) as p2c, \
                     tc.tile_pool(name="ps_st2", bufs=1,
                                  space="PSUM") as psst:
                    st1 = psst.tile([1, c.SQ], F32, name="st1b", tag="st1b")
                    st2 = psst.tile([1, c.SQ], F32, name="st2b", tag="st2b")
                    out_proj("op2", wcrossT, OC, gc_f,
                             lambda j: r(XC[:, j, :]), XC2, st1, st2, p2c)
                    stats_finish(st1, st2, RSB[1], MB[1])

        # =======================================================
        # Phase 3: MLP
        # =======================================================
        with tc.tile_pool(name="p3", bufs=1) as p3:
            XNM = p3.tile([P, c.CH, c.SQ], BF16)
            modulate(XC2, RSB[1], MB[1], shm_f, scm_f, XNM)
            HT = p3.tile([P, c.DHC, c.SQ], BF16)
            with tc.tile_pool(name="ps_m1", bufs=4, space="PSUM") as psm:
                for gj in range(c.DHC):
                    ps = psm.tile([P, c.SQ], F32, name="ps_m1", tag="ps_m1")
                    wt = wk_tile()
                    nc.sync.dma_start(
                        out=wt,
                        in_=wm1T[:, gj * P : (gj + 1) * P].rearrange(
                            "(k p) m -> p k m", p=P
                        ),
                    )
                    for k in range(c.CH):
                        nc.tensor.matmul(
                            ps, wt[:, k, :], XNM[:, k, :],
                            start=(k == 0), stop=(k == c.CH - 1),
                        )
                    nc.scalar.activation(
                        HT[:, gj, :], ps, AF.Gelu_apprx_tanh,
                        bias=BM1[:, gj : gj + 1],
                    )
            with tc.tile_pool(name="ps_m2", bufs=3, space="PSUM") as psm2:
                for j in range(c.CH):
                    ps = psm2.tile([P, c.SQ], F32, name="ps_m2", tag="ps_m2")
                    for kg in range(4):
                        wt = wk_tile()
                        nc.sync.dma_start(
                            out=wt,
                            in_=wm2T[
                                kg * c.CH * P : (kg + 1) * c.CH * P,
                                j * P : (j + 1) * P,
                            ].rearrange("(k p) m -> p k m", p=P),
                        )
                        for k in range(c.CH):
                            gk = kg * c.CH + k
                            nc.tensor.matmul(
                                ps, wt[:, k, :], HT[:, gk, :],
                                start=(gk == 0), stop=(gk == c.DHC - 1),
                            )
                    gt = wf_tile()
                    nc.sync.dma_start(out=gt,
                                      in_=gm_f[j * P : (j + 1) * P, :])
                    t = twb()
                    nc.vector.scalar_tensor_tensor(
                        out=t, in0=ps, scalar=BM2[:, j : j + 1], in1=gt,
                        op0=OP.add, op1=OP.mult,
                    )
                    o = twf()
                    nc.vector.tensor_add(o, t, r(XC2[:, j, :]))
                    nc.sync.dma_start(out=out_d[j * P : (j + 1) * P, :],
                                      in_=o)

    nc.compile()
    return nc


# =======================================================
# Host side
# =======================================================

def host_prep(cfg: Cfg, inputs: dict):
    c = cfg
    f32 = np.float32

    q_x = np.asarray(inputs["q_x"], f32)
    h_content = np.asarray(inputs["h_content"], f32)
    h_obs = np.asarray(inputs["h_obs"], f32)
    t_cond = np.asarray(inputs["t_cond"], f32)
    M_QQ = np.asarray(inputs["M_QQ"], f32)
    M_hyb = np.asarray(inputs["M_hyb"], f32)
    w_ln_self = np.asarray(inputs["w_ln_self"], f32)
    w_qkv = np.asarray(inputs["w_qkv"], f32)
    w_self_out = np.asarray(inputs["w_self_out"], f32)
    w_ln_cross = np.asarray(inputs["w_ln_cross"], f32)
    w_ln_mem = np.asarray(inputs["w_ln_mem"], f32)
    w_qproj = np.asarray(inputs["w_qproj"], f32)
    w_kvproj = np.asarray(inputs["w_kvproj"], f32)
    w_cross_out = np.asarray(inputs["w_cross_out"], f32)
    w_ln_mlp = np.asarray(inputs["w_ln_mlp"], f32)
    w_mlp1 = np.asarray(inputs["w_mlp1"], f32)
    b_mlp1 = np.asarray(inputs["b_mlp1"], f32)
    w_mlp2 = np.asarray(inputs["w_mlp2"], f32)
    b_mlp2 = np.asarray(inputs["b_mlp2"], f32)
    w_ada = np.asarray(inputs["w_ada"], f32)
    b_ada = np.asarray(inputs["b_ada"], f32)

    D, N, HD, SQ = c.D, c.N, c.HD, c.SQ

    # adaLN: fold w_ln into the scale chunks, compute all 9 fields on host
    wada9 = w_ada[: 9 * D].copy()
    bada9 = b_ada[: 9 * D].copy()
    for qd, wl in ((1, w_ln_self), (4, w_ln_cross), (7, w_ln_mlp)):
        wada9[qd * D : (qd + 1) * D] *= wl[:, None]
        bada9[qd * D : (qd + 1) * D] = wl * (1.0 + b_ada[qd * D : (qd + 1) * D])
    ada = (
        t_cond.reshape(c.B * N, c.COND) @ wada9.T + bada9
    ).reshape(c.B, N, 9 * D)

    wqkvT = np.ascontiguousarray(w_qkv.T.astype(BF))
    wselfT = np.ascontiguousarray(w_self_out.T.astype(BF))
    wqT = np.ascontiguousarray(w_qproj.T.astype(BF))
    wkvT = np.ascontiguousarray(w_kvproj.T.astype(BF))
    wcrossT = np.ascontiguousarray(w_cross_out.T.astype(BF))
    wm1T = np.ascontiguousarray(w_mlp1.T.astype(BF))
    wm2T = np.ascontiguousarray(w_mlp2.T.astype(BF))
    bm1_h = np.ascontiguousarray(b_mlp1.reshape(c.DHC, P).T)
    bm2_h = np.ascontiguousarray(b_mlp2.reshape(c.CH, P).T)

    pos = np.arange(N, dtype=f32)
    inv = (10000.0 ** (-np.arange(0, HD, 2, dtype=f32) / HD)).astype(f32)
    freqs = pos[:, None] * inv[None, :]
    cos64 = np.concatenate([np.cos(freqs), np.cos(freqs)], 1)
    s_sgn = np.concatenate([-np.sin(freqs), np.sin(freqs)], 1)
    c_pair = np.ascontiguousarray(np.tile(cos64.T, (2, 1)).astype(f32))
    s_pair = np.ascontiguousarray(np.tile(s_sgn.T, (2, 1)).astype(f32))
    scale = f32(1.0 / np.sqrt(HD))

    def bfc(x):
        return np.ascontiguousarray(x.astype(BF))

    in_maps = []
    for b in range(c.B):
        xb = q_x[b]
        mu_x = xb.mean(-1, keepdims=True)
        rs_x = (1.0 / np.sqrt(xb.var(-1, keepdims=True) + c.eps)).astype(f32)
        ln0 = (xb - mu_x) * rs_x
        xn_self = ln0 * ada[b, :, D : 2 * D] + ada[b, :, 0:D]  # [N, D]

        mem = np.concatenate([h_content[b], h_obs[b]], 0)
        mu_m = mem.mean(-1, keepdims=True)
        rs_m = (1.0 / np.sqrt(mem.var(-1, keepdims=True) + c.eps)).astype(f32)
        hn = ((mem - mu_m) * rs_m) * w_ln_mem[None, :]          # [2N, D]
        hnT = bfc(hn.T)

        mTQQ = np.exp(np.minimum(M_QQ[b].T, 0.0)).astype(f32)   # [keys, q]
        mThyb = np.exp(np.minimum(M_hyb[b].T, 0.0)).astype(f32)  # [2N, N]

        for s in range(2):
            own = np.arange(s * SQ, (s + 1) * SQ)
            rest = np.concatenate(
                [np.arange(0, s * SQ), np.arange((s + 1) * SQ, N)]
            )
            perm = np.concatenate([own, rest]).astype(np.int64)
            po = perm[:SQ]

            mS = mTQQ[perm][:, po]
            mC = mThyb[:N][:, po]
            mO = mThyb[N:][:, po]

            im = {
                "xnT": bfc(xn_self.T[:, perm]),
                "xrT": np.ascontiguousarray(xb.T[:, po]),
                "hnT": hnT,
                "wqkvT": wqkvT, "wselfT": wselfT, "wqT": wqT,
                "wkvT": wkvT, "wcrossT": wcrossT,
                "wm1T": wm1T, "wm2T": wm2T,
                "bm1": bm1_h, "bm2": bm2_h,
                "gs": bfc(ada[b, po, 2 * D : 3 * D].T),
                "shc": bfc(ada[b, po, 3 * D : 4 * D].T),
                "scc": bfc(ada[b, po, 4 * D : 5 * D].T),
                "gc": bfc(ada[b, po, 5 * D : 6 * D].T),
                "shm": bfc(ada[b, po, 6 * D : 7 * D].T),
                "scm": bfc(ada[b, po, 7 * D : 8 * D].T),
                "gm": bfc(ada[b, po, 8 * D : 9 * D].T),
                "cq": bfc(c_pair[:, po] * scale),
                "sq": bfc(s_pair[:, po] * scale),
                "ckS": bfc(c_pair[:, perm]),
                "skS": bfc(s_pair[:, perm]),
                "ckM": bfc(c_pair),
                "skM": bfc(s_pair),
                "mS": bfc(np.concatenate([mS, mS], 1)),
                "mC": bfc(np.concatenate([mC, mC], 1)),
                "mO": bfc(np.concatenate([mO, mO], 1)),
            }
            in_maps.append(im)
    return in_maps


_PROGRAM_CACHE = {}


def get_program(cfg: Cfg):
    key = (cfg.N, cfg.D, cfg.H)
    if key not in _PROGRAM_CACHE:
        _PROGRAM_CACHE[key] = build_program(cfg)
    return _PROGRAM_CACHE[key]


def assemble(cfg: Cfg, results):
    c = cfg
    out = np.zeros((c.B, c.N, c.D), np.float32)
    for b in range(c.B):
        for s in range(2):
            o = results[2 * b + s]["out"]
            out[b, s * c.SQ : (s + 1) * c.SQ, :] = o.T
    return out


def kernel(**inputs) -> np.ndarray:
    cfg = Cfg(mini=False)
    nc = get_program(cfg)
    in_maps = host_prep(cfg, inputs)
    res = bass_utils.run_bass_kernel_spmd(
        nc, in_maps, core_ids=list(range(cfg.n_cores)), trace=False
    )
    return assemble(cfg, res.results)


# revision 18
# speedup vs baseline: 1.8308x; 1.1053x over previous
"""Trainium2 Bass kernel for a DiT-style transformer block (adaLN modulation,
RoPE self-attention with additive rank mask, hybrid cross-attention to
[clean|observed] memory, gated MLP).

Sharding: 8 cores = 4 batches x 2 sequence-halves. Each core computes the
block output for its 512 query tokens of one batch. Per-core token order is
permuted (host side) so the core's own tokens come first.

v2 design notes:
- All matmul operands are bf16 (PE full rate, FWL weight loads, half DMA,
  2x DVE on elementwise ops). PSUM accumulation stays fp32; LN statistics,
  softmax denominators and the residual stream stay fp32.
- Everything that depends only on kernel inputs is precomputed on the host:
  the 9 used adaLN fields (t_cond @ w_ada.T + b_ada), the fully modulated
  self-attention input xn_self, the layernormed memory, exp(mask) in {0,1},
  and scaled RoPE tables.
- Scores for a head pair run as two concurrent K=64 matmuls in disjoint PE
  row groups (partitions 0:64 / 64:128). p@v uses the ones-column trick for
  softmax denominators (v tile has 65 columns; row 64 of o is the denom).
- Activations stay resident in SBUF between phases (no DRAM roundtrip).
- The RoPE rotate-half partition shift is done with 4 batched SBUF-SBUF
  DMAs per projection over all 8 head-pairs at once.
"""

import numpy as np
import ml_dtypes
from contextlib import ExitStack

from concourse import bacc, mybir
import concourse.bass as bass
import concourse.tile as tile
from concourse import bass_utils

F32 = mybir.dt.float32
F32R = mybir.dt.float32r
BF16 = mybir.dt.bfloat16
AF = mybir.ActivationFunctionType
OP = mybir.AluOpType

P = 128
BF = ml_dtypes.bfloat16


class Cfg:
    def __init__(self, mini=False):
        self.B, self.N, self.D, self.H, self.HD = 4, 1024, 1024, 16, 64
        self.COND = 256
        self.DH = 4 * self.D
        self.SQ = self.N // 2            # own query tokens per core
        self.CH = self.D // P            # feature chunks (8)
        self.HH = self.H * self.HD // P  # head-pair chunks (8)
        self.KK = self.N // P            # self key chunks (8)
        self.MKK = 2 * self.N // P       # memory key chunks (16)
        self.DHC = self.DH // P          # mlp hidden chunks (32)
        self.n_cores = 2 * self.B
        self.eps = 1e-5


def build_program(cfg: Cfg):
    c = cfg
    nc = bacc.Bacc(
        "TRN2",
        target_bir_lowering=False,
        debug=False,
        enable_asserts=True,
        num_devices=c.n_cores,
    )

    def din(name, shape, dt=BF16):
        return nc.dram_tensor(name, shape, dt, kind="ExternalInput").ap()

    xnT = din("xnT", [c.D, c.N])            # modulated ln(q_x), feature-major
    xrT = din("xrT", [c.D, c.SQ], F32)      # residual stream (own tokens)
    hnT = din("hnT", [c.D, 2 * c.N])        # normalized memory [clean|obs]
    wqkvT = din("wqkvT", [c.D, 3 * c.D])
    wselfT = din("wselfT", [c.D, c.D])
    wqT = din("wqT", [c.D, c.D])
    wkvT = din("wkvT", [c.D, 2 * c.D])
    wcrossT = din("wcrossT", [c.D, c.D])
    wm1T = din("wm1T", [c.D, c.DH])
    wm2T = din("wm2T", [c.DH, c.D])
    bm1 = din("bm1", [P, c.DHC], F32)
    bm2 = din("bm2", [P, c.CH], F32)
    gs_f = din("gs", [c.D, c.SQ])           # adaLN fields (host-computed)
    shc_f = din("shc", [c.D, c.SQ])
    scc_f = din("scc", [c.D, c.SQ])         # = w_ln_cross*(1+sc_c)
    gc_f = din("gc", [c.D, c.SQ])
    shm_f = din("shm", [c.D, c.SQ])
    scm_f = din("scm", [c.D, c.SQ])
    gm_f = din("gm", [c.D, c.SQ])
    cq_t = din("cq", [P, c.SQ])             # rope tables (scale folded on Q)
    sq_t = din("sq", [P, c.SQ])
    ckS_t = din("ckS", [P, c.N])            # self keys (permuted positions)
    skS_t = din("skS", [P, c.N])
    ckM_t = din("ckM", [P, c.N])            # memory keys (natural positions)
    skM_t = din("skM", [P, c.N])
    i128_d = din("i128", [P, P])            # identity (PSUM mask seed)
    mS_d = din("mS", [c.N, 2 * c.SQ])       # log-mask in {0,-30}, 2-head dup
    mC_d = din("mC", [c.N, 2 * c.SQ])
    mO_d = din("mO", [c.N, 2 * c.SQ])
    out_d = nc.dram_tensor("out", [c.D, c.SQ], F32, kind="ExternalOutput").ap()

    with ExitStack() as ctx:
        tc = ctx.enter_context(tile.TileContext(nc))
        persist = ctx.enter_context(tc.tile_pool(name="persist", bufs=1))
        resid = ctx.enter_context(tc.tile_pool(name="resid", bufs=1))
        ws = ctx.enter_context(tc.tile_pool(name="ws", bufs=1))
        twbp = ctx.enter_context(tc.tile_pool(name="twb", bufs=5))
        twfp = ctx.enter_context(tc.tile_pool(name="twf", bufs=2))
        small = ctx.enter_context(tc.tile_pool(name="small", bufs=1))

        def r(ap):
            return ap.bitcast(F32)

        def twb():
            return twbp.tile([P, c.SQ], BF16, name="twb", tag="twb")

        def twf():
            return twfp.tile([P, c.SQ], F32, name="twf", tag="twf")

        def wk_tile():
            return ws.tile([P, c.CH, P], BF16, name="wk", tag="wk", bufs=4)

        def wv_tile():
            return ws.tile([P, 4, 512], BF16, name="wv", tag="wv", bufs=2)

        def wf_tile():
            return ws.tile([P, c.SQ], BF16, name="wf", tag="wf", bufs=4)

        # ---------- persistent preloads ----------
        CQ = persist.tile([P, c.SQ], BF16)
        nc.sync.dma_start(out=CQ, in_=cq_t)
        SQt = persist.tile([P, c.SQ], BF16)
        nc.sync.dma_start(out=SQt, in_=sq_t)
        BM1 = persist.tile([P, c.DHC], F32)
        nc.sync.dma_start(out=BM1, in_=bm1)
        BM2 = persist.tile([P, c.CH], F32)
        nc.sync.dma_start(out=BM2, in_=bm2)

        I128 = persist.tile([P, P], BF16)
        nc.sync.dma_start(out=I128, in_=i128_d)

        EPS = persist.tile([P, 1], F32)
        nc.vector.memset(EPS, 1e-5)
        ONESB = persist.tile([P, 16], BF16)
        nc.vector.memset(ONESB, 1.0)
        ones_f32 = persist.tile([P, 1], F32)
        nc.vector.memset(ones_f32, 1.0)
        ONEr = persist.tile([P, 1], F32R)
        nc.vector.tensor_copy(ONEr, ones_f32)

        XC = resid.tile([P, c.CH, c.SQ], F32R)   # residual after self-attn
        XC2 = resid.tile([P, c.CH, c.SQ], F32R)  # residual after cross-attn
        XNC = resid.tile([P, c.CH, c.SQ], BF16)  # modulated cross input
        RSB = [
            resid.tile([P, c.SQ], BF16, name=f"RSB{i}", tag=f"RSB{i}")
            for i in range(2)
        ]
        MB = [
            resid.tile([P, c.SQ], BF16, name=f"MB{i}", tag=f"MB{i}")
            for i in range(2)
        ]

        # ---------- helpers ----------
        def shift32(dst, src):
            """dst[p] = src[p xor-32 within each 64-block]."""
            for b in (0, 64):
                nc.sync.dma_start(out=dst[b : b + 32, :],
                                  in_=src[b + 32 : b + 64, :])
                nc.sync.dma_start(out=dst[b + 32 : b + 64, :],
                                  in_=src[b : b + 32, :])

        def qk_proj_rope(tag, wT, col_off, src, src_off, nf, ctab, stab,
                         dst, dst_off):
            """dst[:, hh, dst_off + t] = rope(W[:, cols].T @ src[:, :, t])."""
            nq = nf * c.SQ
            with tc.tile_pool(name=f"z_{tag}", bufs=1) as zpool:
                Z = zpool.tile([P, c.HH, nq], BF16, name="z", tag="z")
                ZS = zpool.tile([P, c.HH, nq], BF16, name="zs", tag="zs")
                with tc.tile_pool(name=f"ps_{tag}", bufs=4,
                                  space="PSUM") as psq:
                    for hh in range(c.HH):
                        wt = wk_tile()
                        nc.sync.dma_start(
                            out=wt,
                            in_=wT[
                                :, col_off + hh * P : col_off + (hh + 1) * P
                            ].rearrange("(k p) m -> p k m", p=P),
                        )
                        for tf in range(nf):
                            ps = psq.tile([P, c.SQ], F32, name="q",
                                          tag="q")
                            for k in range(c.CH):
                                nc.tensor.matmul(
                                    ps, wt[:, k, :],
                                    src[:, k,
                                        src_off + tf * c.SQ :
                                        src_off + (tf + 1) * c.SQ],
                                    start=(k == 0), stop=(k == c.CH - 1),
                                )
                            nc.scalar.activation(
                                Z[:, hh, tf * c.SQ : (tf + 1) * c.SQ], ps,
                                AF.Copy,
                            )
                shift32(ZS, Z)
                for hh in range(c.HH):
                    for tf in range(nf):
                        cs = slice(tf * c.SQ, (tf + 1) * c.SQ)
                        ds = slice(dst_off + tf * c.SQ,
                                   dst_off + (tf + 1) * c.SQ)
                        t1 = twb()
                        nc.vector.tensor_mul(t1, Z[:, hh, cs], ctab[:, cs])
                        t2 = twb()
                        nc.vector.tensor_mul(t2, ZS[:, hh, cs], stab[:, cs])
                        nc.vector.tensor_add(dst[:, hh, ds], t1, t2)

        def v_proj(tag, wT, col_off, src, tt0, ntt, vdst):
            """Token-major value projection with ones column per head."""
            for tt in range(ntt):
                ap = vdst[:, tt0 + tt, :].rearrange(
                    "p (h e) -> p h e", e=65
                )[:, :, 64:65]
                nc.vector.tensor_copy(ap, ONESB[:, 0 : c.H])
            ffw = 512
            nff = (c.H * c.HD) // ffw
            hpf = ffw // 64
            with tc.tile_pool(name=f"ps_{tag}", bufs=8, space="PSUM") as psv:
                for ff in range(nff):
                    pss = [
                        psv.tile([P, ffw], F32, name="v", tag="v")
                        for _ in range(ntt)
                    ]
                    for kg in range(2):
                        wt = wv_tile()
                        nc.sync.dma_start(
                            out=wt,
                            in_=wT[
                                kg * 4 * P : (kg + 1) * 4 * P,
                                col_off + ff * ffw : col_off + (ff + 1) * ffw,
                            ].rearrange("(k p) m -> p k m", p=P),
                        )
                        for k in range(4):
                            gk = kg * 4 + k
                            for tt in range(ntt):
                                nc.tensor.matmul(
                                    pss[tt],
                                    src[:, gk, tt * P : (tt + 1) * P],
                                    wt[:, k, :],
                                    start=(gk == 0), stop=(gk == c.CH - 1),
                                )
                    for tt in range(ntt):
                        ap = vdst[
                            :, tt0 + tt, ff * hpf * 65 : (ff + 1) * hpf * 65
                        ].rearrange("p (h e) -> p h e", e=65)[:, :, 0:64]
                        nc.vector.tensor_copy(ap, pss[tt])

        def mask_fetch(dram_rows, kk):
            """Stream one [P, 2*SQ] mask chunk (rows kk*P..) from DRAM."""
            mt = ws.tile([P, 2 * c.SQ], BF16, name="t_mk", tag="t_mk",
                         bufs=4)
            nc.sync.dma_start(out=mt, in_=dram_rows[kk * P : (kk + 1) * P, :])
            return mt

        def attention(khat, qhat, vtile, masks_fn, n_kk, OST, DEN, ptp):
            """All head pairs; unnormalized output + denominators."""
            with tc.tile_pool(name="ps_oacc", bufs=1, space="PSUM") as opso:
                for gp in range(c.HH // 2):
                    hps = (2 * gp, 2 * gp + 1)
                    ot = {}
                    for i, hp in enumerate(hps):
                        ot[hp] = (
                            opso.tile([65, c.SQ], F32, name=f"o1_{i}",
                                      tag=f"o1_{i}"),
                            opso.tile([65, c.SQ], F32, name=f"o2_{i}",
                                      tag=f"o2_{i}"),
                        )

                    def pv(hp, kk, pt):
                        o1, o2 = ot[hp]
                        h1, h2 = 2 * hp, 2 * hp + 1
                        nc.tensor.matmul(
                            o1, vtile[:, kk, h1 * 65 : (h1 + 1) * 65],
                            pt[:, 0 : c.SQ],
                            start=(kk == 0), stop=(kk == n_kk - 1),
                        )
                        nc.tensor.matmul(
                            o2, vtile[:, kk, h2 * 65 : (h2 + 1) * 65],
                            pt[:, c.SQ : 2 * c.SQ],
                            start=(kk == 0), stop=(kk == n_kk - 1),
                        )

                    pending = []
                    with tc.tile_pool(name="ps_s", bufs=2,
                                      space="PSUM") as pss:
                        for kk in range(n_kk):
                            mt = masks_fn(kk)
                            for hp in hps:
                                ps = pss.tile([P, 2 * c.SQ], F32,
                                              name="ps_s", tag="ps_s")
                                ks = slice(kk * P, (kk + 1) * P)
                                nc.tensor.matmul(
                                    ps[:, 0 : c.SQ], I128, mt[:, 0 : c.SQ],
                                    start=True, stop=False,
                                )
                                nc.tensor.matmul(
                                    ps[:, c.SQ : 2 * c.SQ], I128,
                                    mt[:, c.SQ : 2 * c.SQ],
                                    start=True, stop=False,
                                )
                                nc.tensor.matmul(
                                    ps[:, 0 : c.SQ],
                                    khat[0:64, hp, ks], qhat[0:64, hp, :],
                                    start=False, stop=True,
                                )
                                nc.tensor.matmul(
                                    ps[:, c.SQ : 2 * c.SQ],
                                    khat[64:128, hp, ks],
                                    qhat[64:128, hp, :],
                                    start=False, stop=True,
                                )
                                pt = ptp.tile([P, 2 * c.SQ], BF16,
                                              name="t_p", tag="t_p", bufs=5)
                                nc.scalar.activation(pt, ps, AF.Exp)
                                if len(pending) >= 3:
                                    pv(*pending.pop(0))
                                pending.append((hp, kk, pt))
                        for e in pending:
                            pv(*e)
                    for hp in hps:
                        o1, o2 = ot[hp]
                        st = twf()
                        nc.vector.tensor_copy(st[64:65, :], o1[64:65, :])
                        nc.sync.dma_start(out=DEN[2 * hp : 2 * hp + 1, :],
                                          in_=st[64:65, :])
                        st2 = twf()
                        nc.vector.tensor_copy(st2[64:65, :], o2[64:65, :])
                        nc.sync.dma_start(
                            out=DEN[2 * hp + 1 : 2 * hp + 2, :],
                            in_=st2[64:65, :],
                        )
                        nc.vector.tensor_copy(OST[0:64, hp, :], o1[0:64, :])
                        sthi = twb()
                        nc.vector.tensor_copy(sthi[0:64, :], o2[0:64, :])
                        nc.sync.dma_start(out=OST[64:128, hp, :],
                                          in_=sthi[0:64, :])

        def normalize(OST, DEN, DENI, DENIB, bpool):
            nc.vector.reciprocal(DENI, DEN)
            nc.vector.tensor_copy(DENIB, DENI)
            for hp in range(c.HH):
                d1 = small.tile([1, c.SQ], BF16, name="s_d1", tag="s_d1",
                                bufs=2)
                nc.sync.dma_start(out=d1, in_=DENIB[2 * hp : 2 * hp + 1, :])
                d2 = small.tile([1, c.SQ], BF16, name="s_d2", tag="s_d2",
                                bufs=2)
                nc.sync.dma_start(out=d2,
                                  in_=DENIB[2 * hp + 1 : 2 * hp + 2, :])
                rb = bpool.tile([P, c.SQ], BF16, name="t_rb", tag="t_rb",
                                bufs=2)
                nc.gpsimd.partition_broadcast(rb[0:64, :], d1, channels=64)
                rh = bpool.tile([64, c.SQ], BF16, name="t_rh", tag="t_rh",
                                bufs=2)
                nc.gpsimd.partition_broadcast(rh, d2, channels=64)
                nc.sync.dma_start(out=rb[64:128, :], in_=rh)
                nc.vector.tensor_mul(OST[:, hp, :], OST[:, hp, :], rb)

        def out_proj(tag, wT, osrc, g_dram, xres, xdst, st1, st2, sqpool):
            """xdst[:,j,:] = xres(j) + g_j * (W.T @ o); accumulates LN
            stats of xdst into st1/st2 (PSUM [1, SQ])."""
            with tc.tile_pool(name=f"ps_{tag}", bufs=3, space="PSUM") as pso:
                for j in range(c.CH):
                    ps = pso.tile([P, c.SQ], F32, name="op", tag="op")
                    wt = wk_tile()
                    nc.sync.dma_start(
                        out=wt,
                        in_=wT[:, j * P : (j + 1) * P].rearrange(
                            "(k p) m -> p k m", p=P
                        ),
                    )
                    for hp in range(c.HH):
                        nc.tensor.matmul(
                            ps, wt[:, hp, :], osrc[:, hp, :],
                            start=(hp == 0), stop=(hp == c.HH - 1),
                        )
                    gt = wf_tile()
                    nc.sync.dma_start(out=gt,
                                      in_=g_dram[j * P : (j + 1) * P, :])
                    t = twb()
                    nc.vector.tensor_mul(t, ps, gt)
                    nc.vector.tensor_add(xdst[:, j, :], t, xres(j))
                    sq = sqpool.tile([P, c.SQ], F32R, name="sq", tag="sq",
                                     bufs=2)
                    nc.scalar.activation(sq, r(xdst[:, j, :]), AF.Square)
                    nc.tensor.matmul(
                        st1, ONEr, xdst[:, j, :],
                        start=(j == 0), stop=(j == c.CH - 1),
                    )
                    nc.tensor.matmul(
                        st2, ONEr, sq,
                        start=(j == 0), stop=(j == c.CH - 1),
                    )

        def stats_finish(st1, st2, rs_b, m_b):
            """st1/st2 PSUM [1, SQ] -> broadcast (rstd, mean) bf16 tiles."""
            m = small.tile([1, c.SQ], F32, name="s_a", tag="s_a", bufs=2)
            nc.vector.tensor_scalar_mul(m, st1[0:1, :], 1.0 / c.D)
            e2 = small.tile([1, c.SQ], F32, name="s_b", tag="s_b", bufs=1)
            nc.vector.tensor_scalar_mul(e2, st2[0:1, :], 1.0 / c.D)
            msq = small.tile([1, c.SQ], F32, name="s_c", tag="s_c", bufs=1)
            nc.vector.tensor_mul(msq, m, m)
            var = small.tile([1, c.SQ], F32, name="s_a", tag="s_a", bufs=2)
            nc.vector.tensor_sub(var, e2, msq)
            sd = small.tile([1, c.SQ], F32, name="s_b", tag="s_b", bufs=1)
            nc.scalar.activation(sd, var, AF.Sqrt, bias=EPS[0:1, :])
            rs = small.tile([1, c.SQ], F32, name="s_c", tag="s_c", bufs=1)
            nc.vector.reciprocal(rs, sd)
            rsb = small.tile([1, c.SQ], BF16, name="s_rb", tag="s_rb",
                             bufs=2)
            nc.vector.tensor_copy(rsb, rs)
            mb = small.tile([1, c.SQ], BF16, name="s_mb", tag="s_mb",
                            bufs=2)
            nc.vector.tensor_copy(mb, m)
            nc.gpsimd.partition_broadcast(rs_b, rsb, channels=P)
            nc.gpsimd.partition_broadcast(m_b, mb, channels=P)

        def modulate(xsrc, rs_b, m_b, sh_dram, sc_dram, dst):
            """dst[:,j,:] = (xsrc_j - m)*rs*sc_j + sh_j  (bf16 out)."""
            for j in range(c.CH):
                sct = wf_tile()
                nc.sync.dma_start(out=sct,
                                  in_=sc_dram[j * P : (j + 1) * P, :])
                sht = wf_tile()
                nc.sync.dma_start(out=sht,
                                  in_=sh_dram[j * P : (j + 1) * P, :])
                A = twb()
                nc.vector.tensor_mul(A, rs_b, sct)
                u = twb()
                nc.gpsimd.tensor_sub(u, r(xsrc[:, j, :]), m_b)
                v = twb()
                nc.vector.tensor_mul(v, u, A)
                nc.vector.tensor_add(dst[:, j, :], v, sht)

        # =======================================================
        # Phase 1: self-attention
        # =======================================================
        with tc.tile_pool(name="p1", bufs=1) as p1:
            QHAT = p1.tile([P, c.HH, c.SQ], BF16)
            KHAT = p1.tile([P, c.HH, c.N], BF16)
            VSELF = p1.tile([P, c.KK, c.H * 65], BF16)

            with tc.tile_pool(name="p1a", bufs=1) as p1a:
                XN = p1a.tile([P, c.CH, c.N], BF16)
                nc.sync.dma_start(
                    out=XN, in_=xnT.rearrange("(k p) n -> p k n", p=P)
                )
                CKS = p1a.tile([P, c.N], BF16)
                nc.sync.dma_start(out=CKS, in_=ckS_t)
                SKS = p1a.tile([P, c.N], BF16)
                nc.sync.dma_start(out=SKS, in_=skS_t)
                qk_proj_rope("q1", wqkvT, 0, XN, 0, 1, CQ, SQt, QHAT, 0)
                qk_proj_rope("k1", wqkvT, c.D, XN, 0, 2, CKS, SKS, KHAT, 0)
                v_proj("v1", wqkvT, 2 * c.D, XN, 0, c.KK, VSELF)

            with tc.tile_pool(name="p1o", bufs=1) as p1o:
                OSELF = p1o.tile([P, c.HH, c.SQ], BF16)
                DENS = p1o.tile([2 * c.HH, c.SQ], F32)
                DENSI = p1o.tile([2 * c.HH, c.SQ], F32)
                DENSIB = p1o.tile([2 * c.HH, c.SQ], BF16)
                with tc.tile_pool(name="p1b", bufs=1) as p1b:
                    attention(KHAT, QHAT, VSELF,
                              lambda kk: mask_fetch(mS_d, kk),
                              c.KK, OSELF, DENS, p1b)
                    normalize(OSELF, DENS, DENSI, DENSIB, p1b)

                with tc.tile_pool(name="p1c", bufs=1) as p1c, \
                     tc.tile_pool(name="ps_st1", bufs=1,
                                  space="PSUM") as psst:
                    st1 = psst.tile([1, c.SQ], F32, name="st1", tag="st1")
                    st2 = psst.tile([1, c.SQ], F32, name="st2", tag="st2")

                    def xres1(j):
                        t = twf()
                        nc.sync.dma_start(out=t,
                                          in_=xrT[j * P : (j + 1) * P, :])
                        return t

                    out_proj("op1", wselfT, OSELF, gs_f, xres1, XC,
                             st1, st2, p1c)
                    stats_finish(st1, st2, RSB[0], MB[0])
                    modulate(XC, RSB[0], MB[0], shc_f, scc_f, XNC)

        # =======================================================
        # Phase 2: cross-attention
        # =======================================================
        with tc.tile_pool(name="p2", bufs=1) as p2:
            QC = p2.tile([P, c.HH, c.SQ], BF16)
            KC = p2.tile([P, c.HH, 2 * c.N], BF16)
            VC = p2.tile([P, c.MKK, c.H * 65], BF16)
            CKMt = p2.tile([P, c.N], BF16)
            nc.sync.dma_start(out=CKMt, in_=ckM_t)
            SKMt = p2.tile([P, c.N], BF16)
            nc.sync.dma_start(out=SKMt, in_=skM_t)

            qk_proj_rope("q2", wqT, 0, XNC, 0, 1, CQ, SQt, QC, 0)

            # K/V projection over the 2048 memory tokens, quarter by quarter
            for q in range(4):
                pos0 = (q % 2) * c.SQ
                with tc.tile_pool(name="p2kv", bufs=1) as p2kv:
                    HNQ = p2kv.tile([P, c.CH, c.SQ], BF16)
                    nc.sync.dma_start(
                        out=HNQ,
                        in_=hnT[:, q * c.SQ : (q + 1) * c.SQ].rearrange(
                            "(k p) n -> p k n", p=P
                        ),
                    )
                    qk_proj_rope("k2", wkvT, 0, HNQ, 0, 1,
                                 CKMt[:, pos0 : pos0 + c.SQ],
                                 SKMt[:, pos0 : pos0 + c.SQ],
                                 KC, q * c.SQ)
                    v_proj("v2", wkvT, c.D, HNQ, q * 4, 4, VC)

            with tc.tile_pool(name="p2b", bufs=1) as p2b:
                OC = p2b.tile([P, c.HH, c.SQ], BF16)
                DENC = p2b.tile([2 * c.HH, c.SQ], F32)
                DENCI = p2b.tile([2 * c.HH, c.SQ], F32)
                DENCIB = p2b.tile([2 * c.HH, c.SQ], BF16)

                def cross_mask(kk):
                    if kk < c.KK:
                        return mask_fetch(mC_d, kk)
                    return mask_fetch(mO_d, kk - c.KK)

                attention(KC, QC, VC, cross_mask, c.MKK, OC, DENC, p2b)
                normalize(OC, DENC, DENCI, DENCIB, p2b)

                with tc.tile_pool(name="p2c", bufs=1) as p2c, \
                     tc.tile_pool(name="ps_st2", bufs=1,
                                  space="PSUM") as psst:
                    st1 = psst.tile([1, c.SQ], F32, name="st1b", tag="st1b")
                    st2 = psst.tile([1, c.SQ], F32, name="st2b", tag="st2b")
                    out_proj("op2", wcrossT, OC, gc_f,
                             lambda j: r(XC[:, j, :]), XC2, st1, st2, p2c)
                    stats_finish(st1, st2, RSB[1], MB[1])

        # =======================================================
        # Phase 3: MLP
        # =======================================================
        with tc.tile_pool(name="p3", bufs=1) as p3:
            XNM = p3.tile([P, c.CH, c.SQ], BF16)
            modulate(XC2, RSB[1], MB[1], shm_f, scm_f, XNM)
            HT = p3.tile([P, c.DHC, c.SQ], BF16)
            with tc.tile_pool(name="ps_m1", bufs=4, space="PSUM") as psm:
                for gj in range(c.DHC):
                    ps = psm.tile([P, c.SQ], F32, name="ps_m1", tag="ps_m1")
                    wt = wk_tile()
                    nc.sync.dma_start(
                        out=wt,
                        in_=wm1T[:, gj * P : (gj + 1) * P].rearrange(
                            "(k p) m -> p k m", p=P
                        ),
                    )
                    for k in range(c.CH):
                        nc.tensor.matmul(
                            ps, wt[:, k, :], XNM[:, k, :],
                            start=(k == 0), stop=(k == c.CH - 1),
                        )
                    nc.scalar.activation(
                        HT[:, gj, :], ps, AF.Gelu_apprx_tanh,
                        bias=BM1[:, gj : gj + 1],
                    )
            with tc.tile_pool(name="ps_m2", bufs=3, space="PSUM") as psm2:
                for j in range(c.CH):
                    ps = psm2.tile([P, c.SQ], F32, name="ps_m2", tag="ps_m2")
                    for kg in range(4):
                        wt = wk_tile()
                        nc.sync.dma_start(
                            out=wt,
                            in_=wm2T[
                                kg * c.CH * P : (kg + 1) * c.CH * P,
                                j * P : (j + 1) * P,
                            ].rearrange("(k p) m -> p k m", p=P),
                        )
                        for k in range(c.CH):
                            gk = kg * c.CH + k
                            nc.tensor.matmul(
                                ps, wt[:, k, :], HT[:, gk, :],
                                start=(gk == 0), stop=(gk == c.DHC - 1),
                            )
                    gt = wf_tile()
                    nc.sync.dma_start(out=gt,
                                      in_=gm_f[j * P : (j + 1) * P, :])
                    t = twb()
                    nc.vector.scalar_tensor_tensor(
                        out=t, in0=ps, scalar=BM2[:, j : j + 1], in1=gt,
                        op0=OP.add, op1=OP.mult,
                    )
                    o = twf()
                    nc.vector.tensor_add(o, t, r(XC2[:, j, :]))
                    nc.sync.dma_start(out=out_d[j * P : (j + 1) * P, :],
                                      in_=o)

    nc.compile()
    return nc


# =======================================================
# Host side
# =======================================================

def host_prep(cfg: Cfg, inputs: dict):
    c = cfg
    f32 = np.float32

    q_x = np.asarray(inputs["q_x"], f32)
    h_content = np.asarray(inputs["h_content"], f32)
    h_obs = np.asarray(inputs["h_obs"], f32)
    t_cond = np.asarray(inputs["t_cond"], f32)
    M_QQ = np.asarray(inputs["M_QQ"], f32)
    M_hyb = np.asarray(inputs["M_hyb"], f32)
    w_ln_self = np.asarray(inputs["w_ln_self"], f32)
    w_qkv = np.asarray(inputs["w_qkv"], f32)
    w_self_out = np.asarray(inputs["w_self_out"], f32)
    w_ln_cross = np.asarray(inputs["w_ln_cross"], f32)
    w_ln_mem = np.asarray(inputs["w_ln_mem"], f32)
    w_qproj = np.asarray(inputs["w_qproj"], f32)
    w_kvproj = np.asarray(inputs["w_kvproj"], f32)
    w_cross_out = np.asarray(inputs["w_cross_out"], f32)
    w_ln_mlp = np.asarray(inputs["w_ln_mlp"], f32)
    w_mlp1 = np.asarray(inputs["w_mlp1"], f32)
    b_mlp1 = np.asarray(inputs["b_mlp1"], f32)
    w_mlp2 = np.asarray(inputs["w_mlp2"], f32)
    b_mlp2 = np.asarray(inputs["b_mlp2"], f32)
    w_ada = np.asarray(inputs["w_ada"], f32)
    b_ada = np.asarray(inputs["b_ada"], f32)

    D, N, HD, SQ = c.D, c.N, c.HD, c.SQ

    # adaLN: fold w_ln into the scale chunks, compute all 9 fields on host
    wada9 = w_ada[: 9 * D].copy()
    bada9 = b_ada[: 9 * D].copy()
    for qd, wl in ((1, w_ln_self), (4, w_ln_cross), (7, w_ln_mlp)):
        wada9[qd * D : (qd + 1) * D] *= wl[:, None]
        bada9[qd * D : (qd + 1) * D] = wl * (1.0 + b_ada[qd * D : (qd + 1) * D])
    ada = (
        t_cond.reshape(c.B * N, c.COND) @ wada9.T + bada9
    ).reshape(c.B, N, 9 * D)

    wqkvT = np.ascontiguousarray(w_qkv.T.astype(BF))
    wselfT = np.ascontiguousarray(w_self_out.T.astype(BF))
    wqT = np.ascontiguousarray(w_qproj.T.astype(BF))
    wkvT = np.ascontiguousarray(w_kvproj.T.astype(BF))
    wcrossT = np.ascontiguousarray(w_cross_out.T.astype(BF))
    wm1T = np.ascontiguousarray(w_mlp1.T.astype(BF))
    wm2T = np.ascontiguousarray(w_mlp2.T.astype(BF))
    bm1_h = np.ascontiguousarray(b_mlp1.reshape(c.DHC, P).T)
    bm2_h = np.ascontiguousarray(b_mlp2.reshape(c.CH, P).T)

    pos = np.arange(N, dtype=f32)
    inv = (10000.0 ** (-np.arange(0, HD, 2, dtype=f32) / HD)).astype(f32)
    freqs = pos[:, None] * inv[None, :]
    cos64 = np.concatenate([np.cos(freqs), np.cos(freqs)], 1)
    s_sgn = np.concatenate([-np.sin(freqs), np.sin(freqs)], 1)
    c_pair = np.ascontiguousarray(np.tile(cos64.T, (2, 1)).astype(f32))
    s_pair = np.ascontiguousarray(np.tile(s_sgn.T, (2, 1)).astype(f32))
    scale = f32(1.0 / np.sqrt(HD))

    def bfc(x):
        return np.ascontiguousarray(x.astype(BF))

    in_maps = []
    for b in range(c.B):
        xb = q_x[b]
        mu_x = xb.mean(-1, keepdims=True)
        rs_x = (1.0 / np.sqrt(xb.var(-1, keepdims=True) + c.eps)).astype(f32)
        ln0 = (xb - mu_x) * rs_x
        xn_self = ln0 * ada[b, :, D : 2 * D] + ada[b, :, 0:D]  # [N, D]

        mem = np.concatenate([h_content[b], h_obs[b]], 0)
        mu_m = mem.mean(-1, keepdims=True)
        rs_m = (1.0 / np.sqrt(mem.var(-1, keepdims=True) + c.eps)).astype(f32)
        hn = ((mem - mu_m) * rs_m) * w_ln_mem[None, :]          # [2N, D]
        hnT = bfc(hn.T)

        mTQQ = np.where(M_QQ[b].T < 0.0, f32(-30.0), f32(0.0))   # [keys, q]
        mThyb = np.where(M_hyb[b].T < 0.0, f32(-30.0), f32(0.0))  # [2N, N]

        for s in range(2):
            own = np.arange(s * SQ, (s + 1) * SQ)
            rest = np.concatenate(
                [np.arange(0, s * SQ), np.arange((s + 1) * SQ, N)]
            )
            perm = np.concatenate([own, rest]).astype(np.int64)
            po = perm[:SQ]

            mS = mTQQ[perm][:, po]
            mC = mThyb[:N][:, po]
            mO = mThyb[N:][:, po]

            im = {
                "i128": np.ascontiguousarray(np.eye(P, dtype=BF)),
                "xnT": bfc(xn_self.T[:, perm]),
                "xrT": np.ascontiguousarray(xb.T[:, po]),
                "hnT": hnT,
                "wqkvT": wqkvT, "wselfT": wselfT, "wqT": wqT,
                "wkvT": wkvT, "wcrossT": wcrossT,
                "wm1T": wm1T, "wm2T": wm2T,
                "bm1": bm1_h, "bm2": bm2_h,
                "gs": bfc(ada[b, po, 2 * D : 3 * D].T),
                "shc": bfc(ada[b, po, 3 * D : 4 * D].T),
                "scc": bfc(ada[b, po, 4 * D : 5 * D].T),
                "gc": bfc(ada[b, po, 5 * D : 6 * D].T),
                "shm": bfc(ada[b, po, 6 * D : 7 * D].T),
                "scm": bfc(ada[b, po, 7 * D : 8 * D].T),
                "gm": bfc(ada[b, po, 8 * D : 9 * D].T),
                "cq": bfc(c_pair[:, po] * scale),
                "sq": bfc(s_pair[:, po] * scale),
                "ckS": bfc(c_pair[:, perm]),
                "skS": bfc(s_pair[:, perm]),
                "ckM": bfc(c_pair),
                "skM": bfc(s_pair),
                "mS": bfc(np.concatenate([mS, mS], 1)),
                "mC": bfc(np.concatenate([mC, mC], 1)),
                "mO": bfc(np.concatenate([mO, mO], 1)),
            }
            in_maps.append(im)
    return in_maps


_PROGRAM_CACHE = {}


def get_program(cfg: Cfg):
    key = (cfg.N, cfg.D, cfg.H)
    if key not in _PROGRAM_CACHE:
        _PROGRAM_CACHE[key] = build_program(cfg)
    return _PROGRAM_CACHE[key]


def assemble(cfg: Cfg, results):
    c = cfg
    out = np.zeros((c.B, c.N, c.D), np.float32)
    for b in range(c.B):
        for s in range(2):
            o = results[2 * b + s]["out"]
            out[b, s * c.SQ : (s + 1) * c.SQ, :] = o.T
    return out


def kernel(**inputs) -> np.ndarray:
    cfg = Cfg(mini=False)
    nc = get_program(cfg)
    in_maps = host_prep(cfg, inputs)
    res = bass_utils.run_bass_kernel_spmd(
        nc, in_maps, core_ids=list(range(cfg.n_cores)), trace=False
    )
    return assemble(cfg, res.results)


# revision 21
# speedup vs baseline: 1.9074x; 1.0418x over previous
"""Trainium2 Bass kernel for a DiT-style transformer block (adaLN modulation,
RoPE self-attention with additive rank mask, hybrid cross-attention to
[clean|observed] memory, gated MLP).

Sharding: 8 cores = 4 batches x 2 sequence-halves. Each core computes the
block output for its 512 query tokens of one batch. Per-core token order is
permuted (host side) so the core's own tokens come first.

v2 design notes:
- All matmul operands are bf16 (PE full rate, FWL weight loads, half DMA,
  2x DVE on elementwise ops). PSUM accumulation stays fp32; LN statistics,
  softmax denominators and the residual stream stay fp32.
- Everything that depends only on kernel inputs is precomputed on the host:
  the 9 used adaLN fields (t_cond @ w_ada.T + b_ada), the fully modulated
  self-attention input xn_self, the layernormed memory, exp(mask) in {0,1},
  and scaled RoPE tables.
- Scores for a head pair run as two concurrent K=64 matmuls in disjoint PE
  row groups (partitions 0:64 / 64:128). p@v uses the ones-column trick for
  softmax denominators (v tile has 65 columns; row 64 of o is the denom).
- Activations stay resident in SBUF between phases (no DRAM roundtrip).
- The RoPE rotate-half partition shift is done with 4 batched SBUF-SBUF
  DMAs per projection over all 8 head-pairs at once.
"""

import numpy as np
import ml_dtypes
from contextlib import ExitStack

from concourse import bacc, mybir
import concourse.bass as bass
import concourse.tile as tile
from concourse import bass_utils

F32 = mybir.dt.float32
F32R = mybir.dt.float32r
BF16 = mybir.dt.bfloat16
AF = mybir.ActivationFunctionType
OP = mybir.AluOpType

P = 128
BF = ml_dtypes.bfloat16


class Cfg:
    def __init__(self, mini=False):
        self.B, self.N, self.D, self.H, self.HD = 4, 1024, 1024, 16, 64
        self.COND = 256
        self.DH = 4 * self.D
        self.SQ = self.N // 2            # own query tokens per core
        self.CH = self.D // P            # feature chunks (8)
        self.HH = self.H * self.HD // P  # head-pair chunks (8)
        self.KK = self.N // P            # self key chunks (8)
        self.MKK = 2 * self.N // P       # memory key chunks (16)
        self.DHC = self.DH // P          # mlp hidden chunks (32)
        self.n_cores = 2 * self.B
        self.eps = 1e-5


def build_program(cfg: Cfg):
    c = cfg
    nc = bacc.Bacc(
        "TRN2",
        target_bir_lowering=False,
        debug=False,
        enable_asserts=True,
        num_devices=c.n_cores,
    )

    def din(name, shape, dt=BF16):
        return nc.dram_tensor(name, shape, dt, kind="ExternalInput").ap()

    xnT = din("xnT", [c.D, c.N])            # modulated ln(q_x), feature-major
    xrT = din("xrT", [c.D, c.SQ], F32)      # residual stream (own tokens)
    hnT = din("hnT", [c.D, 2 * c.N])        # normalized memory [clean|obs]
    wqkvT = din("wqkvT", [c.D, 3 * c.D])
    wselfT = din("wselfT", [c.D, c.D])
    wqT = din("wqT", [c.D, c.D])
    wkvT = din("wkvT", [c.D, 2 * c.D])
    wcrossT = din("wcrossT", [c.D, c.D])
    wm1T = din("wm1T", [c.D, c.DH])
    wm2T = din("wm2T", [c.DH, c.D])
    bm1 = din("bm1", [P, c.DHC], F32)
    bm2 = din("bm2", [P, c.CH], F32)
    gs_f = din("gs", [c.D, c.SQ])           # adaLN fields (host-computed)
    shc_f = din("shc", [c.D, c.SQ])
    scc_f = din("scc", [c.D, c.SQ])         # = w_ln_cross*(1+sc_c)
    gc_f = din("gc", [c.D, c.SQ])
    shm_f = din("shm", [c.D, c.SQ])
    scm_f = din("scm", [c.D, c.SQ])
    gm_f = din("gm", [c.D, c.SQ])
    cq_t = din("cq", [P, c.SQ])             # rope tables (scale folded on Q)
    sq_t = din("sq", [P, c.SQ])
    ckS_t = din("ckS", [P, c.N])            # self keys (permuted positions)
    skS_t = din("skS", [P, c.N])
    ckM_t = din("ckM", [P, c.N])            # memory keys (natural positions)
    skM_t = din("skM", [P, c.N])
    i128_d = din("i128", [P, P])            # identity (PSUM mask seed)
    mS_d = din("mS", [c.N, 2 * c.SQ])       # log-mask in {0,-30}, 2-head dup
    mC_d = din("mC", [c.N, 2 * c.SQ])
    mO_d = din("mO", [c.N, 2 * c.SQ])
    out_d = nc.dram_tensor("out", [c.D, c.SQ], F32, kind="ExternalOutput").ap()

    with ExitStack() as ctx:
        tc = ctx.enter_context(tile.TileContext(nc))
        persist = ctx.enter_context(tc.tile_pool(name="persist", bufs=1))
        resid = ctx.enter_context(tc.tile_pool(name="resid", bufs=1))
        ws = ctx.enter_context(tc.tile_pool(name="ws", bufs=1))
        twbp = ctx.enter_context(tc.tile_pool(name="twb", bufs=5))
        twfp = ctx.enter_context(tc.tile_pool(name="twf", bufs=2))
        small = ctx.enter_context(tc.tile_pool(name="small", bufs=1))

        def r(ap):
            return ap.bitcast(F32)

        def twb():
            return twbp.tile([P, c.SQ], BF16, name="twb", tag="twb")

        def twf():
            return twfp.tile([P, c.SQ], F32, name="twf", tag="twf")

        def wk_tile():
            return ws.tile([P, c.CH, P], BF16, name="wk", tag="wk", bufs=4)

        def wv_tile():
            return ws.tile([P, 4, 512], BF16, name="wv", tag="wv", bufs=2)

        def wf_tile():
            return ws.tile([P, c.SQ], BF16, name="wf", tag="wf", bufs=4)

        # ---------- persistent preloads ----------
        CQ = persist.tile([P, c.SQ], BF16)
        nc.sync.dma_start(out=CQ, in_=cq_t)
        SQt = persist.tile([P, c.SQ], BF16)
        nc.sync.dma_start(out=SQt, in_=sq_t)
        BM1 = persist.tile([P, c.DHC], F32)
        nc.sync.dma_start(out=BM1, in_=bm1)
        BM2 = persist.tile([P, c.CH], F32)
        nc.sync.dma_start(out=BM2, in_=bm2)

        I128 = persist.tile([P, P], BF16)
        nc.sync.dma_start(out=I128, in_=i128_d)

        EPS = persist.tile([P, 1], F32)
        nc.vector.memset(EPS, 1e-5)
        ONESB = persist.tile([P, 16], BF16)
        nc.vector.memset(ONESB, 1.0)
        ones_f32 = persist.tile([P, 1], F32)
        nc.vector.memset(ones_f32, 1.0)
        ONEr = persist.tile([P, 1], F32R)
        nc.vector.tensor_copy(ONEr, ones_f32)

        XC = resid.tile([P, c.CH, c.SQ], F32R)   # residual after self-attn
        XC2 = resid.tile([P, c.CH, c.SQ], F32R)  # residual after cross-attn
        XNC = resid.tile([P, c.CH, c.SQ], BF16)  # modulated cross input
        RSB = [
            resid.tile([P, c.SQ], BF16, name=f"RSB{i}", tag=f"RSB{i}")
            for i in range(2)
        ]
        MB = [
            resid.tile([P, c.SQ], BF16, name=f"MB{i}", tag=f"MB{i}")
            for i in range(2)
        ]

        # ---------- helpers ----------
        def shift32(dst, src):
            """dst[p] = src[p xor-32 within each 64-block]."""
            for b in (0, 64):
                nc.sync.dma_start(out=dst[b : b + 32, :],
                                  in_=src[b + 32 : b + 64, :])
                nc.sync.dma_start(out=dst[b + 32 : b + 64, :],
                                  in_=src[b : b + 32, :])

        def qk_proj_rope(tag, wT, col_off, src, src_off, nf, ctab, stab,
                         dst, dst_off):
            """dst[:, hh, dst_off + t] = rope(W[:, cols].T @ src[:, :, t])."""
            nq = nf * c.SQ
            with tc.tile_pool(name=f"z_{tag}", bufs=1) as zpool:
                Z = zpool.tile([P, c.HH, nq], BF16, name="z", tag="z")
                ZS = zpool.tile([P, c.HH, nq], BF16, name="zs", tag="zs")
                with tc.tile_pool(name=f"ps_{tag}", bufs=4,
                                  space="PSUM") as psq:
                    for hh in range(c.HH):
                        wt = wk_tile()
                        nc.sync.dma_start(
                            out=wt,
                            in_=wT[
                                :, col_off + hh * P : col_off + (hh + 1) * P
                            ].rearrange("(k p) m -> p k m", p=P),
                        )
                        for tf in range(nf):
                            ps = psq.tile([P, c.SQ], F32, name="q",
                                          tag="q")
                            for k in range(c.CH):
                                nc.tensor.matmul(
                                    ps, wt[:, k, :],
                                    src[:, k,
                                        src_off + tf * c.SQ :
                                        src_off + (tf + 1) * c.SQ],
                                    start=(k == 0), stop=(k == c.CH - 1),
                                )
                            nc.scalar.activation(
                                Z[:, hh, tf * c.SQ : (tf + 1) * c.SQ], ps,
                                AF.Copy,
                            )
                shift32(ZS, Z)
                for hh in range(c.HH):
                    for tf in range(nf):
                        cs = slice(tf * c.SQ, (tf + 1) * c.SQ)
                        ds = slice(dst_off + tf * c.SQ,
                                   dst_off + (tf + 1) * c.SQ)
                        t1 = twb()
                        nc.vector.tensor_mul(t1, Z[:, hh, cs], ctab[:, cs])
                        t2 = twb()
                        nc.vector.tensor_mul(t2, ZS[:, hh, cs], stab[:, cs])
                        nc.vector.tensor_add(dst[:, hh, ds], t1, t2)

        def v_proj(tag, wT, col_off, src, tt0, ntt, vdst):
            """Token-major value projection with ones column per head."""
            for tt in range(ntt):
                ap = vdst[:, tt0 + tt, :].rearrange(
                    "p (h e) -> p h e", e=65
                )[:, :, 64:65]
                nc.vector.tensor_copy(ap, ONESB[:, 0 : c.H])
            ffw = 512
            nff = (c.H * c.HD) // ffw
            hpf = ffw // 64
            with tc.tile_pool(name=f"ps_{tag}", bufs=8, space="PSUM") as psv:
                for ff in range(nff):
                    pss = [
                        psv.tile([P, ffw], F32, name="v", tag="v")
                        for _ in range(ntt)
                    ]
                    for kg in range(2):
                        wt = wv_tile()
                        nc.sync.dma_start(
                            out=wt,
                            in_=wT[
                                kg * 4 * P : (kg + 1) * 4 * P,
                                col_off + ff * ffw : col_off + (ff + 1) * ffw,
                            ].rearrange("(k p) m -> p k m", p=P),
                        )
                        for k in range(4):
                            gk = kg * 4 + k
                            for tt in range(ntt):
                                nc.tensor.matmul(
                                    pss[tt],
                                    src[:, gk, tt * P : (tt + 1) * P],
                                    wt[:, k, :],
                                    start=(gk == 0), stop=(gk == c.CH - 1),
                                )
                    for tt in range(ntt):
                        ap = vdst[
                            :, tt0 + tt, ff * hpf * 65 : (ff + 1) * hpf * 65
                        ].rearrange("p (h e) -> p h e", e=65)[:, :, 0:64]
                        nc.vector.tensor_copy(ap, pss[tt])

        def mask_fetch(dram_rows, kk):
            """Stream one [P, 2*SQ] mask chunk (rows kk*P..) from DRAM."""
            mt = ws.tile([P, 2 * c.SQ], BF16, name="t_mk", tag="t_mk",
                         bufs=3)
            nc.sync.dma_start(out=mt, in_=dram_rows[kk * P : (kk + 1) * P, :])
            return mt

        def attention(khat, qhat, vtile, masks_fn, n_kk, OST, ptp):
            """All head pairs; per-group softmax normalization."""
            with tc.tile_pool(name="ps_oacc", bufs=1, space="PSUM") as opso:
                for gp in range(c.HH // 2):
                    hps = (2 * gp, 2 * gp + 1)
                    ot = {}
                    for i, hp in enumerate(hps):
                        ot[hp] = (
                            opso.tile([65, c.SQ], F32, name=f"o1_{i}",
                                      tag=f"o1_{i}"),
                            opso.tile([65, c.SQ], F32, name=f"o2_{i}",
                                      tag=f"o2_{i}"),
                        )

                    def pv(hp, kk, pt):
                        o1, o2 = ot[hp]
                        h1, h2 = 2 * hp, 2 * hp + 1
                        nc.tensor.matmul(
                            o1, vtile[:, kk, h1 * 65 : (h1 + 1) * 65],
                            pt[:, 0 : c.SQ],
                            start=(kk == 0), stop=(kk == n_kk - 1),
                        )
                        nc.tensor.matmul(
                            o2, vtile[:, kk, h2 * 65 : (h2 + 1) * 65],
                            pt[:, c.SQ : 2 * c.SQ],
                            start=(kk == 0), stop=(kk == n_kk - 1),
                        )

                    pending = []
                    with tc.tile_pool(name="ps_s", bufs=2,
                                      space="PSUM") as pss:
                        for kk in range(n_kk):
                            mt = masks_fn(kk)
                            for hp in hps:
                                ps = pss.tile([P, 2 * c.SQ], F32,
                                              name="ps_s", tag="ps_s")
                                ks = slice(kk * P, (kk + 1) * P)
                                nc.tensor.matmul(
                                    ps[:, 0 : c.SQ], I128, mt[:, 0 : c.SQ],
                                    start=True, stop=False,
                                )
                                nc.tensor.matmul(
                                    ps[:, c.SQ : 2 * c.SQ], I128,
                                    mt[:, c.SQ : 2 * c.SQ],
                                    start=True, stop=False,
                                )
                                nc.tensor.matmul(
                                    ps[:, 0 : c.SQ],
                                    khat[0:64, hp, ks], qhat[0:64, hp, :],
                                    start=False, stop=True,
                                )
                                nc.tensor.matmul(
                                    ps[:, c.SQ : 2 * c.SQ],
                                    khat[64:128, hp, ks],
                                    qhat[64:128, hp, :],
                                    start=False, stop=True,
                                )
                                pt = ptp.tile([P, 2 * c.SQ], BF16,
                                              name="t_p", tag="t_p", bufs=5)
                                nc.scalar.activation(pt, ps, AF.Exp)
                                if len(pending) >= 3:
                                    pv(*pending.pop(0))
                                pending.append((hp, kk, pt))
                        for e in pending:
                            pv(*e)
                    deng = ptp.tile([4, c.SQ], F32, name="deng",
                                    tag="deng", bufs=2)
                    dengib = ptp.tile([4, c.SQ], BF16, name="dengib",
                                      tag="dengib", bufs=2)
                    for i, hp in enumerate(hps):
                        o1, o2 = ot[hp]
                        st = twf()
                        nc.vector.tensor_copy(st[64:65, :], o1[64:65, :])
                        nc.sync.dma_start(out=deng[2 * i : 2 * i + 1, :],
                                          in_=st[64:65, :])
                        st2 = twf()
                        nc.vector.tensor_copy(st2[64:65, :], o2[64:65, :])
                        nc.sync.dma_start(
                            out=deng[2 * i + 1 : 2 * i + 2, :],
                            in_=st2[64:65, :],
                        )
                        nc.vector.tensor_copy(OST[0:64, hp, :], o1[0:64, :])
                        sthi = twb()
                        nc.vector.tensor_copy(sthi[0:64, :], o2[0:64, :])
                        nc.sync.dma_start(out=OST[64:128, hp, :],
                                          in_=sthi[0:64, :])
                    # normalize this group's heads while the next group runs
                    nc.vector.reciprocal(deng, deng)
                    nc.vector.tensor_copy(dengib, deng)
                    for i, hp in enumerate(hps):
                        d1 = small.tile([1, c.SQ], BF16, name="s_d1",
                                        tag="s_d1", bufs=2)
                        nc.sync.dma_start(
                            out=d1, in_=dengib[2 * i : 2 * i + 1, :]
                        )
                        d2 = small.tile([1, c.SQ], BF16, name="s_d2",
                                        tag="s_d2", bufs=2)
                        nc.sync.dma_start(
                            out=d2, in_=dengib[2 * i + 1 : 2 * i + 2, :]
                        )
                        rb = ptp.tile([P, c.SQ], BF16, name="t_rb",
                                      tag="t_rb", bufs=2)
                        nc.gpsimd.partition_broadcast(rb[0:64, :], d1,
                                                      channels=64)
                        rh = ptp.tile([64, c.SQ], BF16, name="t_rh",
                                      tag="t_rh", bufs=2)
                        nc.gpsimd.partition_broadcast(rh, d2, channels=64)
                        nc.sync.dma_start(out=rb[64:128, :], in_=rh)
                        nc.vector.tensor_mul(OST[:, hp, :], OST[:, hp, :],
                                             rb)

        def out_proj(tag, wT, osrc, g_dram, xres, xdst, st1, st2):
            """xdst[:,j,:] = xres(j) + g_j * (W.T @ o); accumulates LN
            stats of xdst into st1/st2 (PSUM [1, SQ])."""
            with tc.tile_pool(name=f"ps_{tag}", bufs=3, space="PSUM") as pso:
                for j in range(c.CH):
                    ps = pso.tile([P, c.SQ], F32, name="op", tag="op")
                    wt = wk_tile()
                    nc.sync.dma_start(
                        out=wt,
                        in_=wT[:, j * P : (j + 1) * P].rearrange(
                            "(k p) m -> p k m", p=P
                        ),
                    )
                    for hp in range(c.HH):
                        nc.tensor.matmul(
                            ps, wt[:, hp, :], osrc[:, hp, :],
                            start=(hp == 0), stop=(hp == c.HH - 1),
                        )
                    gt = wf_tile()
                    nc.sync.dma_start(out=gt,
                                      in_=g_dram[j * P : (j + 1) * P, :])
                    t = twb()
                    nc.vector.tensor_mul(t, ps, gt)
                    nc.vector.tensor_add(xdst[:, j, :], t, xres(j))
                    sq = ws.tile([P, c.SQ], F32R, name="sq", tag="sq",
                                 bufs=2)
                    nc.scalar.activation(sq, r(xdst[:, j, :]), AF.Square)
                    nc.tensor.matmul(
                        st1, ONEr, xdst[:, j, :],
                        start=(j == 0), stop=(j == c.CH - 1),
                    )
                    nc.tensor.matmul(
                        st2, ONEr, sq,
                        start=(j == 0), stop=(j == c.CH - 1),
                    )

        def stats_finish(st1, st2, rs_b, m_b):
            """st1/st2 PSUM [1, SQ] -> broadcast (rstd, mean) bf16 tiles."""
            m = small.tile([1, c.SQ], F32, name="s_a", tag="s_a", bufs=2)
            nc.vector.tensor_scalar_mul(m, st1[0:1, :], 1.0 / c.D)
            e2 = small.tile([1, c.SQ], F32, name="s_b", tag="s_b", bufs=1)
            nc.vector.tensor_scalar_mul(e2, st2[0:1, :], 1.0 / c.D)
            msq = small.tile([1, c.SQ], F32, name="s_c", tag="s_c", bufs=1)
            nc.vector.tensor_mul(msq, m, m)
            var = small.tile([1, c.SQ], F32, name="s_a", tag="s_a", bufs=2)
            nc.vector.tensor_sub(var, e2, msq)
            sd = small.tile([1, c.SQ], F32, name="s_b", tag="s_b", bufs=1)
            nc.scalar.activation(sd, var, AF.Sqrt, bias=EPS[0:1, :])
            rs = small.tile([1, c.SQ], F32, name="s_c", tag="s_c", bufs=1)
            nc.vector.reciprocal(rs, sd)
            rsb = small.tile([1, c.SQ], BF16, name="s_rb", tag="s_rb",
                             bufs=2)
            nc.vector.tensor_copy(rsb, rs)
            mb = small.tile([1, c.SQ], BF16, name="s_mb", tag="s_mb",
                            bufs=2)
            nc.vector.tensor_copy(mb, m)
            nc.gpsimd.partition_broadcast(rs_b, rsb, channels=P)
            nc.gpsimd.partition_broadcast(m_b, mb, channels=P)

        def modulate(xsrc, rs_b, m_b, sh_dram, sc_dram, dst):
            """dst[:,j,:] = (xsrc_j - m)*rs*sc_j + sh_j  (bf16 out)."""
            for j in range(c.CH):
                sct = wf_tile()
                nc.sync.dma_start(out=sct,
                                  in_=sc_dram[j * P : (j + 1) * P, :])
                sht = wf_tile()
                nc.sync.dma_start(out=sht,
                                  in_=sh_dram[j * P : (j + 1) * P, :])
                A = twb()
                nc.vector.tensor_mul(A, rs_b, sct)
                u = twb()
                nc.gpsimd.tensor_sub(u, r(xsrc[:, j, :]), m_b)
                v = twb()
                nc.vector.tensor_mul(v, u, A)
                nc.vector.tensor_add(dst[:, j, :], v, sht)

        # =======================================================
        # Phase 1: self-attention
        # =======================================================
        with tc.tile_pool(name="p1o", bufs=1) as p1o:
            OSELF = p1o.tile([P, c.HH, c.SQ], BF16)

            with tc.tile_pool(name="p1big", bufs=1) as p1big:
                QHAT = p1big.tile([P, c.HH, c.SQ], BF16)
                KHAT = p1big.tile([P, c.HH, c.N], BF16)
                VSELF = p1big.tile([P, c.KK, c.H * 65], BF16)

                with tc.tile_pool(name="p1a", bufs=1) as p1a:
                    XN = p1a.tile([P, c.CH, c.N], BF16)
                    nc.sync.dma_start(
                        out=XN, in_=xnT.rearrange("(k p) n -> p k n", p=P)
                    )
                    CKS = p1a.tile([P, c.N], BF16)
                    nc.sync.dma_start(out=CKS, in_=ckS_t)
                    SKS = p1a.tile([P, c.N], BF16)
                    nc.sync.dma_start(out=SKS, in_=skS_t)
                    qk_proj_rope("k1", wqkvT, c.D, XN, 0, 2, CKS, SKS,
                                 KHAT, 0)
                    v_proj("v1", wqkvT, 2 * c.D, XN, 0, c.KK, VSELF)
                    qk_proj_rope("q1", wqkvT, 0, XN, 0, 1, CQ, SQt, QHAT, 0)

                with tc.tile_pool(name="p1b", bufs=1) as p1b:
                    attention(KHAT, QHAT, VSELF,
                              lambda kk: mask_fetch(mS_d, kk),
                              c.KK, OSELF, p1b)

            with tc.tile_pool(name="ps_st1", bufs=1, space="PSUM") as psst:
                st1 = psst.tile([1, c.SQ], F32, name="st1", tag="st1")
                st2 = psst.tile([1, c.SQ], F32, name="st2", tag="st2")

                def xres1(j):
                    t = twf()
                    nc.sync.dma_start(out=t,
                                      in_=xrT[j * P : (j + 1) * P, :])
                    return t

                out_proj("op1", wselfT, OSELF, gs_f, xres1, XC, st1, st2)
                stats_finish(st1, st2, RSB[0], MB[0])
                modulate(XC, RSB[0], MB[0], shc_f, scc_f, XNC)

        # =======================================================
        # Phase 2: cross-attention
        # =======================================================
        with tc.tile_pool(name="p2", bufs=1) as p2:
            QC = p2.tile([P, c.HH, c.SQ], BF16)
            KC = p2.tile([P, c.HH, 2 * c.N], BF16)
            VC = p2.tile([P, c.MKK, c.H * 65], BF16)
            CKMt = p2.tile([P, c.N], BF16)
            nc.sync.dma_start(out=CKMt, in_=ckM_t)
            SKMt = p2.tile([P, c.N], BF16)
            nc.sync.dma_start(out=SKMt, in_=skM_t)

            # K/V projection over the 2048 memory tokens, quarter by quarter
            # (emitted before the Q projection: K/V depend only on inputs,
            # so they overlap the phase-1 tail on the PE)
            for q in range(4):
                pos0 = (q % 2) * c.SQ
                with tc.tile_pool(name="p2kv", bufs=1) as p2kv:
                    HNQ = p2kv.tile([P, c.CH, c.SQ], BF16)
                    nc.sync.dma_start(
                        out=HNQ,
                        in_=hnT[:, q * c.SQ : (q + 1) * c.SQ].rearrange(
                            "(k p) n -> p k n", p=P
                        ),
                    )
                    qk_proj_rope("k2", wkvT, 0, HNQ, 0, 1,
                                 CKMt[:, pos0 : pos0 + c.SQ],
                                 SKMt[:, pos0 : pos0 + c.SQ],
                                 KC, q * c.SQ)
                    v_proj("v2", wkvT, c.D, HNQ, q * 4, 4, VC)

            qk_proj_rope("q2", wqT, 0, XNC, 0, 1, CQ, SQt, QC, 0)

            with tc.tile_pool(name="p2b", bufs=1) as p2b:
                OC = p2b.tile([P, c.HH, c.SQ], BF16)

                def cross_mask(kk):
                    if kk < c.KK:
                        return mask_fetch(mC_d, kk)
                    return mask_fetch(mO_d, kk - c.KK)

                attention(KC, QC, VC, cross_mask, c.MKK, OC, p2b)

                with tc.tile_pool(name="ps_st2", bufs=1,
                                  space="PSUM") as psst:
                    st1 = psst.tile([1, c.SQ], F32, name="st1b", tag="st1b")
                    st2 = psst.tile([1, c.SQ], F32, name="st2b", tag="st2b")
                    out_proj("op2", wcrossT, OC, gc_f,
                             lambda j: r(XC[:, j, :]), XC2, st1, st2)
                    stats_finish(st1, st2, RSB[1], MB[1])

        # =======================================================
        # Phase 3: MLP
        # =======================================================
        with tc.tile_pool(name="p3", bufs=1) as p3:
            XNM = p3.tile([P, c.CH, c.SQ], BF16)
            modulate(XC2, RSB[1], MB[1], shm_f, scm_f, XNM)
            HT = p3.tile([P, c.DHC, c.SQ], BF16)
            with tc.tile_pool(name="ps_m1", bufs=4, space="PSUM") as psm:
                for gj in range(c.DHC):
                    ps = psm.tile([P, c.SQ], F32, name="ps_m1", tag="ps_m1")
                    wt = wk_tile()
                    nc.sync.dma_start(
                        out=wt,
                        in_=wm1T[:, gj * P : (gj + 1) * P].rearrange(
                            "(k p) m -> p k m", p=P
                        ),
                    )
                    for k in range(c.CH):
                        nc.tensor.matmul(
                            ps, wt[:, k, :], XNM[:, k, :],
                            start=(k == 0), stop=(k == c.CH - 1),
                        )
                    nc.scalar.activation(
                        HT[:, gj, :], ps, AF.Gelu_apprx_tanh,
                        bias=BM1[:, gj : gj + 1],
                    )
            with tc.tile_pool(name="ps_m2", bufs=3, space="PSUM") as psm2:
                for j in range(c.CH):
                    ps = psm2.tile([P, c.SQ], F32, name="ps_m2", tag="ps_m2")
                    for kg in range(4):
                        wt = wk_tile()
                        nc.sync.dma_start(
                            out=wt,
                            in_=wm2T[
                                kg * c.CH * P : (kg + 1) * c.CH * P,
                                j * P : (j + 1) * P,
                            ].rearrange("(k p) m -> p k m", p=P),
                        )
                        for k in range(c.CH):
                            gk = kg * c.CH + k
                            nc.tensor.matmul(
                                ps, wt[:, k, :], HT[:, gk, :],
                                start=(gk == 0), stop=(gk == c.DHC - 1),
                            )
                    gt = wf_tile()
                    nc.sync.dma_start(out=gt,
                                      in_=gm_f[j * P : (j + 1) * P, :])
                    t = twb()
                    nc.vector.scalar_tensor_tensor(
                        out=t, in0=ps, scalar=BM2[:, j : j + 1], in1=gt,
                        op0=OP.add, op1=OP.mult,
                    )
                    o = twf()
                    nc.vector.tensor_add(o, t, r(XC2[:, j, :]))
                    nc.sync.dma_start(out=out_d[j * P : (j + 1) * P, :],
                                      in_=o)

    nc.compile()
    return nc


# =======================================================
# Host side
# =======================================================

def host_prep(cfg: Cfg, inputs: dict):
    c = cfg
    f32 = np.float32

    q_x = np.asarray(inputs["q_x"], f32)
    h_content = np.asarray(inputs["h_content"], f32)
    h_obs = np.asarray(inputs["h_obs"], f32)
    t_cond = np.asarray(inputs["t_cond"], f32)
    M_QQ = np.asarray(inputs["M_QQ"], f32)
    M_hyb = np.asarray(inputs["M_hyb"], f32)
    w_ln_self = np.asarray(inputs["w_ln_self"], f32)
    w_qkv = np.asarray(inputs["w_qkv"], f32)
    w_self_out = np.asarray(inputs["w_self_out"], f32)
    w_ln_cross = np.asarray(inputs["w_ln_cross"], f32)
    w_ln_mem = np.asarray(inputs["w_ln_mem"], f32)
    w_qproj = np.asarray(inputs["w_qproj"], f32)
    w_kvproj = np.asarray(inputs["w_kvproj"], f32)
    w_cross_out = np.asarray(inputs["w_cross_out"], f32)
    w_ln_mlp = np.asarray(inputs["w_ln_mlp"], f32)
    w_mlp1 = np.asarray(inputs["w_mlp1"], f32)
    b_mlp1 = np.asarray(inputs["b_mlp1"], f32)
    w_mlp2 = np.asarray(inputs["w_mlp2"], f32)
    b_mlp2 = np.asarray(inputs["b_mlp2"], f32)
    w_ada = np.asarray(inputs["w_ada"], f32)
    b_ada = np.asarray(inputs["b_ada"], f32)

    D, N, HD, SQ = c.D, c.N, c.HD, c.SQ

    # adaLN: fold w_ln into the scale chunks, compute all 9 fields on host
    wada9 = w_ada[: 9 * D].copy()
    bada9 = b_ada[: 9 * D].copy()
    for qd, wl in ((1, w_ln_self), (4, w_ln_cross), (7, w_ln_mlp)):
        wada9[qd * D : (qd + 1) * D] *= wl[:, None]
        bada9[qd * D : (qd + 1) * D] = wl * (1.0 + b_ada[qd * D : (qd + 1) * D])
    ada = (
        t_cond.reshape(c.B * N, c.COND) @ wada9.T + bada9
    ).reshape(c.B, N, 9 * D)

    wqkvT = np.ascontiguousarray(w_qkv.T.astype(BF))
    wselfT = np.ascontiguousarray(w_self_out.T.astype(BF))
    wqT = np.ascontiguousarray(w_qproj.T.astype(BF))
    wkvT = np.ascontiguousarray(w_kvproj.T.astype(BF))
    wcrossT = np.ascontiguousarray(w_cross_out.T.astype(BF))
    wm1T = np.ascontiguousarray(w_mlp1.T.astype(BF))
    wm2T = np.ascontiguousarray(w_mlp2.T.astype(BF))
    bm1_h = np.ascontiguousarray(b_mlp1.reshape(c.DHC, P).T)
    bm2_h = np.ascontiguousarray(b_mlp2.reshape(c.CH, P).T)

    pos = np.arange(N, dtype=f32)
    inv = (10000.0 ** (-np.arange(0, HD, 2, dtype=f32) / HD)).astype(f32)
    freqs = pos[:, None] * inv[None, :]
    cos64 = np.concatenate([np.cos(freqs), np.cos(freqs)], 1)
    s_sgn = np.concatenate([-np.sin(freqs), np.sin(freqs)], 1)
    c_pair = np.ascontiguousarray(np.tile(cos64.T, (2, 1)).astype(f32))
    s_pair = np.ascontiguousarray(np.tile(s_sgn.T, (2, 1)).astype(f32))
    scale = f32(1.0 / np.sqrt(HD))

    def bfc(x):
        return np.ascontiguousarray(x.astype(BF))

    in_maps = []
    for b in range(c.B):
        xb = q_x[b]
        mu_x = xb.mean(-1, keepdims=True)
        rs_x = (1.0 / np.sqrt(xb.var(-1, keepdims=True) + c.eps)).astype(f32)
        ln0 = (xb - mu_x) * rs_x
        xn_self = ln0 * ada[b, :, D : 2 * D] + ada[b, :, 0:D]  # [N, D]

        mem = np.concatenate([h_content[b], h_obs[b]], 0)
        mu_m = mem.mean(-1, keepdims=True)
        rs_m = (1.0 / np.sqrt(mem.var(-1, keepdims=True) + c.eps)).astype(f32)
        hn = ((mem - mu_m) * rs_m) * w_ln_mem[None, :]          # [2N, D]
        hnT = bfc(hn.T)

        mTQQ = np.where(M_QQ[b].T < 0.0, f32(-30.0), f32(0.0))   # [keys, q]
        mThyb = np.where(M_hyb[b].T < 0.0, f32(-30.0), f32(0.0))  # [2N, N]

        for s in range(2):
            own = np.arange(s * SQ, (s + 1) * SQ)
            rest = np.concatenate(
                [np.arange(0, s * SQ), np.arange((s + 1) * SQ, N)]
            )
            perm = np.concatenate([own, rest]).astype(np.int64)
            po = perm[:SQ]

            mS = mTQQ[perm][:, po]
            mC = mThyb[:N][:, po]
            mO = mThyb[N:][:, po]

            im = {
                "i128": np.ascontiguousarray(np.eye(P, dtype=BF)),
                "xnT": bfc(xn_self.T[:, perm]),
                "xrT": np.ascontiguousarray(xb.T[:, po]),
                "hnT": hnT,
                "wqkvT": wqkvT, "wselfT": wselfT, "wqT": wqT,
                "wkvT": wkvT, "wcrossT": wcrossT,
                "wm1T": wm1T, "wm2T": wm2T,
                "bm1": bm1_h, "bm2": bm2_h,
                "gs": bfc(ada[b, po, 2 * D : 3 * D].T),
                "shc": bfc(ada[b, po, 3 * D : 4 * D].T),
                "scc": bfc(ada[b, po, 4 * D : 5 * D].T),
                "gc": bfc(ada[b, po, 5 * D : 6 * D].T),
                "shm": bfc(ada[b, po, 6 * D : 7 * D].T),
                "scm": bfc(ada[b, po, 7 * D : 8 * D].T),
                "gm": bfc(ada[b, po, 8 * D : 9 * D].T),
                "cq": bfc(c_pair[:, po] * scale),
                "sq": bfc(s_pair[:, po] * scale),
                "ckS": bfc(c_pair[:, perm]),
                "skS": bfc(s_pair[:, perm]),
                "ckM": bfc(c_pair),
                "skM": bfc(s_pair),
                "mS": bfc(np.concatenate([mS, mS], 1)),
                "mC": bfc(np.concatenate([mC, mC], 1)),
                "mO": bfc(np.concatenate([mO, mO], 1)),
            }
            in_maps.append(im)
    return in_maps


_PROGRAM_CACHE = {}


def get_program(cfg: Cfg):
    key = (cfg.N, cfg.D, cfg.H)
    if key not in _PROGRAM_CACHE:
        _PROGRAM_CACHE[key] = build_program(cfg)
    return _PROGRAM_CACHE[key]


def assemble(cfg: Cfg, results):
    c = cfg
    out = np.zeros((c.B, c.N, c.D), np.float32)
    for b in range(c.B):
        for s in range(2):
            o = results[2 * b + s]["out"]
            out[b, s * c.SQ : (s + 1) * c.SQ, :] = o.T
    return out


def kernel(**inputs) -> np.ndarray:
    cfg = Cfg(mini=False)
    nc = get_program(cfg)
    in_maps = host_prep(cfg, inputs)
    res = bass_utils.run_bass_kernel_spmd(
        nc, in_maps, core_ids=list(range(cfg.n_cores)), trace=False
    )
    return assemble(cfg, res.results)


# revision 26
# speedup vs baseline: 1.9947x; 1.0458x over previous
"""Trainium2 Bass kernel for a DiT-style transformer block (adaLN modulation,
RoPE self-attention with additive rank mask, hybrid cross-attention to
[clean|observed] memory, gated MLP).

Sharding: 8 cores = 4 batches x 2 sequence-halves. Each core computes the
block output for its 512 query tokens of one batch. Per-core token order is
permuted (host side) so the core's own tokens come first.

v2 design notes:
- All matmul operands are bf16 (PE full rate, FWL weight loads, half DMA,
  2x DVE on elementwise ops). PSUM accumulation stays fp32; LN statistics,
  softmax denominators and the residual stream stay fp32.
- Everything that depends only on kernel inputs is precomputed on the host:
  the 9 used adaLN fields (t_cond @ w_ada.T + b_ada), the fully modulated
  self-attention input xn_self, the layernormed memory, exp(mask) in {0,1},
  and scaled RoPE tables.
- Scores for a head pair run as two concurrent K=64 matmuls in disjoint PE
  row groups (partitions 0:64 / 64:128). p@v uses the ones-column trick for
  softmax denominators (v tile has 65 columns; row 64 of o is the denom).
- Activations stay resident in SBUF between phases (no DRAM roundtrip).
- The RoPE rotate-half partition shift is done with 4 batched SBUF-SBUF
  DMAs per projection over all 8 head-pairs at once.
"""

import numpy as np
import ml_dtypes
from contextlib import ExitStack

from concourse import bacc, mybir
import concourse.bass as bass
import concourse.tile as tile
from concourse import bass_utils

F32 = mybir.dt.float32
F32R = mybir.dt.float32r
BF16 = mybir.dt.bfloat16
AF = mybir.ActivationFunctionType
OP = mybir.AluOpType

P = 128
BF = ml_dtypes.bfloat16


class Cfg:
    def __init__(self, mini=False):
        self.B, self.N, self.D, self.H, self.HD = 4, 1024, 1024, 16, 64
        self.COND = 256
        self.DH = 4 * self.D
        self.SQ = self.N // 2            # own query tokens per core
        self.CH = self.D // P            # feature chunks (8)
        self.HH = self.H * self.HD // P  # head-pair chunks (8)
        self.KK = self.N // P            # self key chunks (8)
        self.MKK = 2 * self.N // P       # memory key chunks (16)
        self.DHC = self.DH // P          # mlp hidden chunks (32)
        self.n_cores = 2 * self.B
        self.eps = 1e-5


def build_program(cfg: Cfg):
    c = cfg
    nc = bacc.Bacc(
        "TRN2",
        target_bir_lowering=False,
        debug=False,
        enable_asserts=True,
        num_devices=c.n_cores,
    )

    def din(name, shape, dt=BF16):
        return nc.dram_tensor(name, shape, dt, kind="ExternalInput").ap()

    xnT = din("xnT", [c.D, c.N])            # modulated ln(q_x), feature-major
    xrT = din("xrT", [c.D, c.SQ], F32)      # residual stream (own tokens)
    hnT = din("hnT", [c.D, 2 * c.N])        # normalized memory [clean|obs]
    wqkvT = din("wqkvT", [c.D, 3 * c.D])
    wselfT = din("wselfT", [c.D, c.D])
    wqT = din("wqT", [c.D, c.D])
    wkvT = din("wkvT", [c.D, 2 * c.D])
    wcrossT = din("wcrossT", [c.D, c.D])
    wm1T = din("wm1T", [c.D, c.DH])
    wm2T = din("wm2T", [c.DH, c.D])
    bm1 = din("bm1", [P, c.DHC], F32)
    bm2 = din("bm2", [P, c.CH], F32)
    gs_f = din("gs", [c.D, c.SQ])           # adaLN fields (host-computed)
    shc_f = din("shc", [c.D, c.SQ])
    scc_f = din("scc", [c.D, c.SQ])         # = w_ln_cross*(1+sc_c)
    gc_f = din("gc", [c.D, c.SQ])
    shm_f = din("shm", [c.D, c.SQ])
    scm_f = din("scm", [c.D, c.SQ])
    gm_f = din("gm", [c.D, c.SQ])
    cq_t = din("cq", [P, c.SQ])             # rope tables (scale folded on Q)
    sq_t = din("sq", [P, c.SQ])
    ckS_t = din("ckS", [P, c.N])            # self keys (permuted positions)
    skS_t = din("skS", [P, c.N])
    ckM_t = din("ckM", [P, c.N])            # memory keys (natural positions)
    skM_t = din("skM", [P, c.N])
    i128_d = din("i128", [P, P])            # identity (PSUM mask seed)
    mS_d = din("mS", [c.N, 2 * c.SQ])       # log-mask in {0,-30}, 2-head dup
    mC_d = din("mC", [c.N, 2 * c.SQ])
    mO_d = din("mO", [c.N, 2 * c.SQ])
    out_d = nc.dram_tensor("out", [c.D, c.SQ], F32, kind="ExternalOutput").ap()

    with ExitStack() as ctx:
        tc = ctx.enter_context(tile.TileContext(nc))
        persist = ctx.enter_context(tc.tile_pool(name="persist", bufs=1))
        resid = ctx.enter_context(tc.tile_pool(name="resid", bufs=1))
        ws = ctx.enter_context(tc.tile_pool(name="ws", bufs=1))
        twbp = ctx.enter_context(tc.tile_pool(name="twb", bufs=4))
        twfp = ctx.enter_context(tc.tile_pool(name="twf", bufs=2))
        small = ctx.enter_context(tc.tile_pool(name="small", bufs=1))

        def r(ap):
            return ap.bitcast(F32)

        def twb():
            return twbp.tile([P, c.SQ], BF16, name="twb", tag="twb")

        def twf():
            return twfp.tile([P, c.SQ], F32, name="twf", tag="twf")

        def wk_tile():
            return ws.tile([P, c.CH, P], BF16, name="wk", tag="wk", bufs=4)

        def wv_tile():
            return ws.tile([P, 4, 512], BF16, name="wv", tag="wv", bufs=2)

        def wf_tile():
            return ws.tile([P, c.SQ], BF16, name="wf", tag="wf", bufs=3)

        # ---------- persistent preloads ----------
        CQ = persist.tile([P, c.SQ], BF16)
        nc.sync.dma_start(out=CQ, in_=cq_t)
        SQt = persist.tile([P, c.SQ], BF16)
        nc.sync.dma_start(out=SQt, in_=sq_t)
        BM1 = persist.tile([P, c.DHC], F32)
        nc.sync.dma_start(out=BM1, in_=bm1)
        BM2 = persist.tile([P, c.CH], F32)
        nc.sync.dma_start(out=BM2, in_=bm2)

        I128 = persist.tile([P, P], BF16)
        nc.sync.dma_start(out=I128, in_=i128_d)

        EPS = persist.tile([P, 1], F32)
        nc.vector.memset(EPS, 1e-5)
        ONESB = persist.tile([P, 16], BF16)
        nc.vector.memset(ONESB, 1.0)
        ones_f32 = persist.tile([P, 1], F32)
        nc.vector.memset(ones_f32, 1.0)
        ONEr = persist.tile([P, 1], F32R)
        nc.vector.tensor_copy(ONEr, ones_f32)

        XC = resid.tile([P, c.CH, c.SQ], F32R)   # residual after self-attn
        XC2 = resid.tile([P, c.CH, c.SQ], F32R)  # residual after cross-attn
        XNC = resid.tile([P, c.CH, c.SQ], BF16)  # modulated cross input
        RSB = [
            resid.tile([P, c.SQ], BF16, name=f"RSB{i}", tag=f"RSB{i}")
            for i in range(2)
        ]
        MB = [
            resid.tile([P, c.SQ], BF16, name=f"MB{i}", tag=f"MB{i}")
            for i in range(2)
        ]

        # ---------- helpers ----------
        def shift32(dst, src):
            """dst[p] = src[p xor-32 within each 64-block]."""
            for b in (0, 64):
                nc.sync.dma_start(out=dst[b : b + 32, :],
                                  in_=src[b + 32 : b + 64, :])
                nc.sync.dma_start(out=dst[b + 32 : b + 64, :],
                                  in_=src[b : b + 32, :])

        def qk_proj_rope(tag, wT, col_off, src, src_off, nf, ctab, stab,
                         dst, dst_off):
            """dst[:, hh, dst_off + t] = rope(W[:, cols].T @ src[:, :, t])."""
            nq = nf * c.SQ
            with tc.tile_pool(name=f"z_{tag}", bufs=1) as zpool:
                Z = zpool.tile([P, c.HH, nq], BF16, name="z", tag="z")
                ZS = zpool.tile([P, c.HH, nq], BF16, name="zs", tag="zs")
                with tc.tile_pool(name=f"ps_{tag}", bufs=4,
                                  space="PSUM") as psq:
                    for hh in range(c.HH):
                        wt = wk_tile()
                        nc.sync.dma_start(
                            out=wt,
                            in_=wT[
                                :, col_off + hh * P : col_off + (hh + 1) * P
                            ].rearrange("(k p) m -> p k m", p=P),
                        )
                        for tf in range(nf):
                            ps = psq.tile([P, c.SQ], F32, name="q",
                                          tag="q")
                            for k in range(c.CH):
                                nc.tensor.matmul(
                                    ps, wt[:, k, :],
                                    src[:, k,
                                        src_off + tf * c.SQ :
                                        src_off + (tf + 1) * c.SQ],
                                    start=(k == 0), stop=(k == c.CH - 1),
                                )
                            nc.scalar.activation(
                                Z[:, hh, tf * c.SQ : (tf + 1) * c.SQ], ps,
                                AF.Copy,
                            )
                shift32(ZS, Z)
                for hh in range(c.HH):
                    for tf in range(nf):
                        cs = slice(tf * c.SQ, (tf + 1) * c.SQ)
                        ds = slice(dst_off + tf * c.SQ,
                                   dst_off + (tf + 1) * c.SQ)
                        t1 = twb()
                        nc.vector.tensor_mul(t1, Z[:, hh, cs], ctab[:, cs])
                        t2 = twb()
                        nc.vector.tensor_mul(t2, ZS[:, hh, cs], stab[:, cs])
                        nc.vector.tensor_add(dst[:, hh, ds], t1, t2)

        def v_proj(tag, wT, col_off, src, tt0, ntt, vdst):
            """Token-major value projection with ones column per head."""
            for tt in range(ntt):
                ap = vdst[:, tt0 + tt, :].rearrange(
                    "p (h e) -> p h e", e=65
                )[:, :, 64:65]
                nc.vector.tensor_copy(ap, ONESB[:, 0 : c.H])
            ffw = 512
            nff = (c.H * c.HD) // ffw
            hpf = ffw // 64
            with tc.tile_pool(name=f"ps_{tag}", bufs=8, space="PSUM") as psv:
                for ff in range(nff):
                    pss = [
                        psv.tile([P, ffw], F32, name="v", tag="v")
                        for _ in range(ntt)
                    ]
                    for kg in range(2):
                        wt = wv_tile()
                        nc.sync.dma_start(
                            out=wt,
                            in_=wT[
                                kg * 4 * P : (kg + 1) * 4 * P,
                                col_off + ff * ffw : col_off + (ff + 1) * ffw,
                            ].rearrange("(k p) m -> p k m", p=P),
                        )
                        for k in range(4):
                            gk = kg * 4 + k
                            for tt in range(ntt):
                                nc.tensor.matmul(
                                    pss[tt],
                                    src[:, gk, tt * P : (tt + 1) * P],
                                    wt[:, k, :],
                                    start=(gk == 0), stop=(gk == c.CH - 1),
                                )
                    for tt in range(ntt):
                        ap = vdst[
                            :, tt0 + tt, ff * hpf * 65 : (ff + 1) * hpf * 65
                        ].rearrange("p (h e) -> p h e", e=65)[:, :, 0:64]
                        nc.vector.tensor_copy(ap, pss[tt])

        def mask_fetch(dram_rows, kk):
            """Stream one [P, 2*SQ] mask chunk (rows kk*P..) from DRAM."""
            mt = ws.tile([P, 2 * c.SQ], BF16, name="t_mk", tag="t_mk",
                         bufs=3)
            nc.sync.dma_start(out=mt, in_=dram_rows[kk * P : (kk + 1) * P, :])
            return mt

        def attention(khat, qhat, vtile, masks_fn, n_kk, OST, ptp):
            """All head pairs; per-group softmax normalization."""
            with tc.tile_pool(name="ps_oacc", bufs=1, space="PSUM") as opso:
                for gp in range(c.HH // 2):
                    hps = (2 * gp, 2 * gp + 1)
                    ot = {}
                    for i, hp in enumerate(hps):
                        ot[hp] = (
                            opso.tile([65, c.SQ], F32, name=f"o1_{i}",
                                      tag=f"o1_{i}"),
                            opso.tile([65, c.SQ], F32, name=f"o2_{i}",
                                      tag=f"o2_{i}"),
                        )

                    def pv(hp, kk, pt):
                        o1, o2 = ot[hp]
                        h1, h2 = 2 * hp, 2 * hp + 1
                        nc.tensor.matmul(
                            o1, vtile[:, kk, h1 * 65 : (h1 + 1) * 65],
                            pt[:, 0 : c.SQ],
                            start=(kk == 0), stop=(kk == n_kk - 1),
                        )
                        nc.tensor.matmul(
                            o2, vtile[:, kk, h2 * 65 : (h2 + 1) * 65],
                            pt[:, c.SQ : 2 * c.SQ],
                            start=(kk == 0), stop=(kk == n_kk - 1),
                        )

                    pending = []
                    with tc.tile_pool(name="ps_s", bufs=2,
                                      space="PSUM") as pss:
                        for kk in range(n_kk):
                            mt = masks_fn(kk)
                            for hp in hps:
                                ps = pss.tile([P, 2 * c.SQ], F32,
                                              name="ps_s", tag="ps_s")
                                ks = slice(kk * P, (kk + 1) * P)
                                nc.tensor.matmul(
                                    ps[:, 0 : c.SQ], I128, mt[:, 0 : c.SQ],
                                    start=True, stop=False,
                                )
                                nc.tensor.matmul(
                                    ps[:, c.SQ : 2 * c.SQ], I128,
                                    mt[:, c.SQ : 2 * c.SQ],
                                    start=True, stop=False,
                                )
                                nc.tensor.matmul(
                                    ps[:, 0 : c.SQ],
                                    khat[0:64, hp, ks], qhat[0:64, hp, :],
                                    start=False, stop=True,
                                )
                                nc.tensor.matmul(
                                    ps[:, c.SQ : 2 * c.SQ],
                                    khat[64:128, hp, ks],
                                    qhat[64:128, hp, :],
                                    start=False, stop=True,
                                )
                                pt = ptp.tile([P, 2 * c.SQ], BF16,
                                              name="t_p", tag="t_p", bufs=5)
                                nc.scalar.activation(pt, ps, AF.Exp)
                                if len(pending) >= 3:
                                    pv(*pending.pop(0))
                                pending.append((hp, kk, pt))
                        for e in pending:
                            pv(*e)
                    deng = ptp.tile([4, c.SQ], F32, name="deng",
                                    tag="deng", bufs=2)
                    dengib = ptp.tile([4, c.SQ], BF16, name="dengib",
                                      tag="dengib", bufs=2)
                    for i, hp in enumerate(hps):
                        o1, o2 = ot[hp]
                        st = twf()
                        nc.vector.tensor_copy(st[64:65, :], o1[64:65, :])
                        nc.sync.dma_start(out=deng[2 * i : 2 * i + 1, :],
                                          in_=st[64:65, :])
                        st2 = twf()
                        nc.vector.tensor_copy(st2[64:65, :], o2[64:65, :])
                        nc.sync.dma_start(
                            out=deng[2 * i + 1 : 2 * i + 2, :],
                            in_=st2[64:65, :],
                        )
                        nc.vector.tensor_copy(OST[0:64, hp, :], o1[0:64, :])
                        sthi = twb()
                        nc.vector.tensor_copy(sthi[0:64, :], o2[0:64, :])
                        nc.sync.dma_start(out=OST[64:128, hp, :],
                                          in_=sthi[0:64, :])
                    # normalize this group's heads while the next group runs
                    nc.vector.reciprocal(deng, deng)
                    nc.vector.tensor_copy(dengib, deng)
                    for i, hp in enumerate(hps):
                        d1 = small.tile([1, c.SQ], BF16, name="s_d1",
                                        tag="s_d1", bufs=1)
                        nc.sync.dma_start(
                            out=d1, in_=dengib[2 * i : 2 * i + 1, :]
                        )
                        d2 = small.tile([1, c.SQ], BF16, name="s_d2",
                                        tag="s_d2", bufs=1)
                        nc.sync.dma_start(
                            out=d2, in_=dengib[2 * i + 1 : 2 * i + 2, :]
                        )
                        rb = ptp.tile([P, c.SQ], BF16, name="t_rb",
                                      tag="t_rb", bufs=2)
                        nc.gpsimd.partition_broadcast(rb[0:64, :], d1,
                                                      channels=64)
                        rh = ptp.tile([64, c.SQ], BF16, name="t_rh",
                                      tag="t_rh", bufs=2)
                        nc.gpsimd.partition_broadcast(rh, d2, channels=64)
                        nc.sync.dma_start(out=rb[64:128, :], in_=rh)
                        nc.vector.tensor_mul(OST[:, hp, :], OST[:, hp, :],
                                             rb)

        def out_proj(tag, wT, osrc, g_dram, xres, xdst, st1, st2):
            """xdst[:,j,:] = xres(j) + g_j * (W.T @ o); accumulates LN
            stats of xdst into st1/st2 (PSUM [1, SQ])."""
            with tc.tile_pool(name=f"ps_{tag}", bufs=3, space="PSUM") as pso:
                for j in range(c.CH):
                    ps = pso.tile([P, c.SQ], F32, name="op", tag="op")
                    wt = wk_tile()
                    nc.sync.dma_start(
                        out=wt,
                        in_=wT[:, j * P : (j + 1) * P].rearrange(
                            "(k p) m -> p k m", p=P
                        ),
                    )
                    for hp in range(c.HH):
                        nc.tensor.matmul(
                            ps, wt[:, hp, :], osrc[:, hp, :],
                            start=(hp == 0), stop=(hp == c.HH - 1),
                        )
                    gt = wf_tile()
                    nc.sync.dma_start(out=gt,
                                      in_=g_dram[j * P : (j + 1) * P, :])
                    t = twb()
                    nc.vector.tensor_mul(t, ps, gt)
                    nc.vector.tensor_add(xdst[:, j, :], t, xres(j))
                    sq = ws.tile([P, c.SQ], F32R, name="sq", tag="sq",
                                 bufs=2)
                    nc.scalar.activation(sq, r(xdst[:, j, :]), AF.Square)
                    nc.tensor.matmul(
                        st1, ONEr, xdst[:, j, :],
                        start=(j == 0), stop=(j == c.CH - 1),
                    )
                    nc.tensor.matmul(
                        st2, ONEr, sq,
                        start=(j == 0), stop=(j == c.CH - 1),
                    )

        def stats_finish(st1, st2, rs_b, m_b):
            """st1/st2 PSUM [1, SQ] -> broadcast (rstd, mean) bf16 tiles."""
            m = small.tile([1, c.SQ], F32, name="s_a", tag="s_a", bufs=2)
            nc.vector.tensor_scalar_mul(m, st1[0:1, :], 1.0 / c.D)
            e2 = small.tile([1, c.SQ], F32, name="s_b", tag="s_b", bufs=1)
            nc.vector.tensor_scalar_mul(e2, st2[0:1, :], 1.0 / c.D)
            msq = small.tile([1, c.SQ], F32, name="s_c", tag="s_c", bufs=1)
            nc.vector.tensor_mul(msq, m, m)
            var = small.tile([1, c.SQ], F32, name="s_a", tag="s_a", bufs=2)
            nc.vector.tensor_sub(var, e2, msq)
            sd = small.tile([1, c.SQ], F32, name="s_b", tag="s_b", bufs=1)
            nc.scalar.activation(sd, var, AF.Sqrt, bias=EPS[0:1, :])
            rs = small.tile([1, c.SQ], F32, name="s_c", tag="s_c", bufs=1)
            nc.vector.reciprocal(rs, sd)
            rsb = small.tile([1, c.SQ], BF16, name="s_rb", tag="s_rb",
                             bufs=2)
            nc.vector.tensor_copy(rsb, rs)
            mb = small.tile([1, c.SQ], BF16, name="s_mb", tag="s_mb",
                            bufs=2)
            nc.vector.tensor_copy(mb, m)
            nc.gpsimd.partition_broadcast(rs_b, rsb, channels=P)
            nc.gpsimd.partition_broadcast(m_b, mb, channels=P)

        def modulate(xsrc, rs_b, m_b, sh_dram, sc_dram, dst):
            """dst[:,j,:] = (xsrc_j - m)*rs*sc_j + sh_j  (bf16 out)."""
            for j in range(c.CH):
                sct = wf_tile()
                nc.sync.dma_start(out=sct,
                                  in_=sc_dram[j * P : (j + 1) * P, :])
                sht = wf_tile()
                nc.sync.dma_start(out=sht,
                                  in_=sh_dram[j * P : (j + 1) * P, :])
                A = twb()
                nc.vector.tensor_mul(A, rs_b, sct)
                u = twb()
                nc.vector.tensor_sub(u, r(xsrc[:, j, :]), m_b)
                v = twb()
                nc.vector.tensor_mul(v, u, A)
                nc.vector.tensor_add(dst[:, j, :], v, sht)

        # =======================================================
        # Phase 1: self-attention
        # =======================================================
        with tc.tile_pool(name="p1o", bufs=1) as p1o:
            OSELF = p1o.tile([P, c.HH, c.SQ], BF16)

            with tc.tile_pool(name="p1big", bufs=1) as p1big:
                QHAT = p1big.tile([P, c.HH, c.SQ], BF16)
                KHAT = p1big.tile([P, c.HH, c.N], BF16)
                VSELF = p1big.tile([P, c.KK, c.H * 65], BF16)

                with tc.tile_pool(name="p1a", bufs=1) as p1a:
                    XN = p1a.tile([P, c.CH, c.N], BF16)
                    for j in range(c.CH):
                        nc.sync.dma_start(
                            out=XN[:, j, :],
                            in_=xnT[j * P : (j + 1) * P, :],
                        )
                    CKS = p1a.tile([P, c.N], BF16)
                    nc.sync.dma_start(out=CKS, in_=ckS_t)
                    SKS = p1a.tile([P, c.N], BF16)
                    nc.sync.dma_start(out=SKS, in_=skS_t)
                    qk_proj_rope("k1", wqkvT, c.D, XN, 0, 2, CKS, SKS,
                                 KHAT, 0)
                    v_proj("v1", wqkvT, 2 * c.D, XN, 0, c.KK, VSELF)
                    qk_proj_rope("q1", wqkvT, 0, XN, 0, 1, CQ, SQt, QHAT, 0)

                with tc.tile_pool(name="p1b", bufs=1) as p1b:
                    attention(KHAT, QHAT, VSELF,
                              lambda kk: mask_fetch(mS_d, kk),
                              c.KK, OSELF, p1b)

            with tc.tile_pool(name="ps_st1", bufs=1, space="PSUM") as psst:
                st1 = psst.tile([1, c.SQ], F32, name="st1", tag="st1")
                st2 = psst.tile([1, c.SQ], F32, name="st2", tag="st2")

                def xres1(j):
                    t = twf()
                    nc.sync.dma_start(out=t,
                                      in_=xrT[j * P : (j + 1) * P, :])
                    return t

                out_proj("op1", wselfT, OSELF, gs_f, xres1, XC, st1, st2)
                stats_finish(st1, st2, RSB[0], MB[0])
                modulate(XC, RSB[0], MB[0], shc_f, scc_f, XNC)

        # =======================================================
        # Phase 2: cross-attention
        # =======================================================
        with tc.tile_pool(name="p2", bufs=1) as p2:
            QC = p2.tile([P, c.HH, c.SQ], BF16)
            KC = p2.tile([P, c.HH, 2 * c.N], BF16)
            VC = p2.tile([P, c.MKK, c.H * 65], BF16)
            CKMt = p2.tile([P, c.N], BF16)
            nc.sync.dma_start(out=CKMt, in_=ckM_t)
            SKMt = p2.tile([P, c.N], BF16)
            nc.sync.dma_start(out=SKMt, in_=skM_t)

            # K/V projection over the 2048 memory tokens, quarter by quarter
            # (emitted before the Q projection: K/V depend only on inputs,
            # so they overlap the phase-1 tail on the PE)
            p2hn_cm = tc.tile_pool(name="p2hn", bufs=1)
            p2hn = p2hn_cm.__enter__()
            for q in range(4):
                pos0 = (q % 2) * c.SQ
                HNQ = p2hn.tile([P, c.CH, c.SQ], BF16, name="HNQ",
                                tag="HNQ", bufs=2)
                for j in range(c.CH):
                    nc.sync.dma_start(
                        out=HNQ[:, j, :],
                        in_=hnT[j * P : (j + 1) * P,
                                q * c.SQ : (q + 1) * c.SQ],
                    )
                qk_proj_rope("k2", wkvT, 0, HNQ, 0, 1,
                             CKMt[:, pos0 : pos0 + c.SQ],
                             SKMt[:, pos0 : pos0 + c.SQ],
                             KC, q * c.SQ)
                v_proj("v2", wkvT, c.D, HNQ, q * 4, 4, VC)
            p2hn_cm.__exit__(None, None, None)

            qk_proj_rope("q2", wqT, 0, XNC, 0, 1, CQ, SQt, QC, 0)

            with tc.tile_pool(name="p2b", bufs=1) as p2b:
                OC = p2b.tile([P, c.HH, c.SQ], BF16)

                def cross_mask(kk):
                    if kk < c.KK:
                        return mask_fetch(mC_d, kk)
                    return mask_fetch(mO_d, kk - c.KK)

                attention(KC, QC, VC, cross_mask, c.MKK, OC, p2b)

                with tc.tile_pool(name="ps_st2", bufs=1,
                                  space="PSUM") as psst:
                    st1 = psst.tile([1, c.SQ], F32, name="st1b", tag="st1b")
                    st2 = psst.tile([1, c.SQ], F32, name="st2b", tag="st2b")
                    out_proj("op2", wcrossT, OC, gc_f,
                             lambda j: r(XC[:, j, :]), XC2, st1, st2)
                    stats_finish(st1, st2, RSB[1], MB[1])

        # =======================================================
        # Phase 3: MLP
        # =======================================================
        with tc.tile_pool(name="p3", bufs=1) as p3:
            XNM = p3.tile([P, c.CH, c.SQ], BF16)
            modulate(XC2, RSB[1], MB[1], shm_f, scm_f, XNM)
            HT = p3.tile([P, c.DHC, c.SQ], BF16)
            with tc.tile_pool(name="ps_m1", bufs=4, space="PSUM") as psm:
                for gj in range(c.DHC):
                    ps = psm.tile([P, c.SQ], F32, name="ps_m1", tag="ps_m1")
                    wt = wk_tile()
                    nc.sync.dma_start(
                        out=wt,
                        in_=wm1T[:, gj * P : (gj + 1) * P].rearrange(
                            "(k p) m -> p k m", p=P
                        ),
                    )
                    for k in range(c.CH):
                        nc.tensor.matmul(
                            ps, wt[:, k, :], XNM[:, k, :],
                            start=(k == 0), stop=(k == c.CH - 1),
                        )
                    nc.scalar.activation(
                        HT[:, gj, :], ps, AF.Gelu_apprx_tanh,
                        bias=BM1[:, gj : gj + 1],
                    )
            with tc.tile_pool(name="ps_m2", bufs=3, space="PSUM") as psm2:
                for j in range(c.CH):
                    ps = psm2.tile([P, c.SQ], F32, name="ps_m2", tag="ps_m2")
                    for kg in range(4):
                        wt = wk_tile()
                        nc.sync.dma_start(
                            out=wt,
                            in_=wm2T[
                                kg * c.CH * P : (kg + 1) * c.CH * P,
                                j * P : (j + 1) * P,
                            ].rearrange("(k p) m -> p k m", p=P),
                        )
                        for k in range(c.CH):
                            gk = kg * c.CH + k
                            nc.tensor.matmul(
                                ps, wt[:, k, :], HT[:, gk, :],
                                start=(gk == 0), stop=(gk == c.DHC - 1),
                            )
                    gt = wf_tile()
                    nc.sync.dma_start(out=gt,
                                      in_=gm_f[j * P : (j + 1) * P, :])
                    t = twb()
                    nc.vector.scalar_tensor_tensor(
                        out=t, in0=ps, scalar=BM2[:, j : j + 1], in1=gt,
                        op0=OP.add, op1=OP.mult,
                    )
                    o = twf()
                    nc.vector.tensor_add(o, t, r(XC2[:, j, :]))
                    nc.sync.dma_start(out=out_d[j * P : (j + 1) * P, :],
                                      in_=o)

    nc.compile()
    return nc


# =======================================================
# Host side
# =======================================================

def host_prep(cfg: Cfg, inputs: dict):
    c = cfg
    f32 = np.float32

    q_x = np.asarray(inputs["q_x"], f32)
    h_content = np.asarray(inputs["h_content"], f32)
    h_obs = np.asarray(inputs["h_obs"], f32)
    t_cond = np.asarray(inputs["t_cond"], f32)
    M_QQ = np.asarray(inputs["M_QQ"], f32)
    M_hyb = np.asarray(inputs["M_hyb"], f32)
    w_ln_self = np.asarray(inputs["w_ln_self"], f32)
    w_qkv = np.asarray(inputs["w_qkv"], f32)
    w_self_out = np.asarray(inputs["w_self_out"], f32)
    w_ln_cross = np.asarray(inputs["w_ln_cross"], f32)
    w_ln_mem = np.asarray(inputs["w_ln_mem"], f32)
    w_qproj = np.asarray(inputs["w_qproj"], f32)
    w_kvproj = np.asarray(inputs["w_kvproj"], f32)
    w_cross_out = np.asarray(inputs["w_cross_out"], f32)
    w_ln_mlp = np.asarray(inputs["w_ln_mlp"], f32)
    w_mlp1 = np.asarray(inputs["w_mlp1"], f32)
    b_mlp1 = np.asarray(inputs["b_mlp1"], f32)
    w_mlp2 = np.asarray(inputs["w_mlp2"], f32)
    b_mlp2 = np.asarray(inputs["b_mlp2"], f32)
    w_ada = np.asarray(inputs["w_ada"], f32)
    b_ada = np.asarray(inputs["b_ada"], f32)

    D, N, HD, SQ = c.D, c.N, c.HD, c.SQ

    # adaLN: fold w_ln into the scale chunks, compute all 9 fields on host
    wada9 = w_ada[: 9 * D].copy()
    bada9 = b_ada[: 9 * D].copy()
    for qd, wl in ((1, w_ln_self), (4, w_ln_cross), (7, w_ln_mlp)):
        wada9[qd * D : (qd + 1) * D] *= wl[:, None]
        bada9[qd * D : (qd + 1) * D] = wl * (1.0 + b_ada[qd * D : (qd + 1) * D])
    ada = (
        t_cond.reshape(c.B * N, c.COND) @ wada9.T + bada9
    ).reshape(c.B, N, 9 * D)

    wqkvT = np.ascontiguousarray(w_qkv.T.astype(BF))
    wselfT = np.ascontiguousarray(w_self_out.T.astype(BF))
    wqT = np.ascontiguousarray(w_qproj.T.astype(BF))
    wkvT = np.ascontiguousarray(w_kvproj.T.astype(BF))
    wcrossT = np.ascontiguousarray(w_cross_out.T.astype(BF))
    wm1T = np.ascontiguousarray(w_mlp1.T.astype(BF))
    wm2T = np.ascontiguousarray(w_mlp2.T.astype(BF))
    bm1_h = np.ascontiguousarray(b_mlp1.reshape(c.DHC, P).T)
    bm2_h = np.ascontiguousarray(b_mlp2.reshape(c.CH, P).T)

    pos = np.arange(N, dtype=f32)
    inv = (10000.0 ** (-np.arange(0, HD, 2, dtype=f32) / HD)).astype(f32)
    freqs = pos[:, None] * inv[None, :]
    cos64 = np.concatenate([np.cos(freqs), np.cos(freqs)], 1)
    s_sgn = np.concatenate([-np.sin(freqs), np.sin(freqs)], 1)
    c_pair = np.ascontiguousarray(np.tile(cos64.T, (2, 1)).astype(f32))
    s_pair = np.ascontiguousarray(np.tile(s_sgn.T, (2, 1)).astype(f32))
    scale = f32(1.0 / np.sqrt(HD))

    def bfc(x):
        return np.ascontiguousarray(x.astype(BF))

    in_maps = []
    for b in range(c.B):
        xb = q_x[b]
        mu_x = xb.mean(-1, keepdims=True)
        rs_x = (1.0 / np.sqrt(xb.var(-1, keepdims=True) + c.eps)).astype(f32)
        ln0 = (xb - mu_x) * rs_x
        xn_self = ln0 * ada[b, :, D : 2 * D] + ada[b, :, 0:D]  # [N, D]

        mem = np.concatenate([h_content[b], h_obs[b]], 0)
        mu_m = mem.mean(-1, keepdims=True)
        rs_m = (1.0 / np.sqrt(mem.var(-1, keepdims=True) + c.eps)).astype(f32)
        hn = ((mem - mu_m) * rs_m) * w_ln_mem[None, :]          # [2N, D]
        hnT = bfc(hn.T)

        mTQQ = np.where(M_QQ[b].T < 0.0, f32(-30.0), f32(0.0))   # [keys, q]
        mThyb = np.where(M_hyb[b].T < 0.0, f32(-30.0), f32(0.0))  # [2N, N]

        for s in range(2):
            own = np.arange(s * SQ, (s + 1) * SQ)
            rest = np.concatenate(
                [np.arange(0, s * SQ), np.arange((s + 1) * SQ, N)]
            )
            perm = np.concatenate([own, rest]).astype(np.int64)
            po = perm[:SQ]

            mS = mTQQ[perm][:, po]
            mC = mThyb[:N][:, po]
            mO = mThyb[N:][:, po]

            im = {
                "i128": np.ascontiguousarray(np.eye(P, dtype=BF)),
                "xnT": bfc(xn_self.T[:, perm]),
                "xrT": np.ascontiguousarray(xb.T[:, po]),
                "hnT": hnT,
                "wqkvT": wqkvT, "wselfT": wselfT, "wqT": wqT,
                "wkvT": wkvT, "wcrossT": wcrossT,
                "wm1T": wm1T, "wm2T": wm2T,
                "bm1": bm1_h, "bm2": bm2_h,
                "gs": bfc(ada[b, po, 2 * D : 3 * D].T),
                "shc": bfc(ada[b, po, 3 * D : 4 * D].T),
                "scc": bfc(ada[b, po, 4 * D : 5 * D].T),
                "gc": bfc(ada[b, po, 5 * D : 6 * D].T),
                "shm": bfc(ada[b, po, 6 * D : 7 * D].T),
                "scm": bfc(ada[b, po, 7 * D : 8 * D].T),
                "gm": bfc(ada[b, po, 8 * D : 9 * D].T),
                "cq": bfc(c_pair[:, po] * scale),
                "sq": bfc(s_pair[:, po] * scale),
                "ckS": bfc(c_pair[:, perm]),
                "skS": bfc(s_pair[:, perm]),
                "ckM": bfc(c_pair),
                "skM": bfc(s_pair),
                "mS": bfc(np.concatenate([mS, mS], 1)),
                "mC": bfc(np.concatenate([mC, mC], 1)),
                "mO": bfc(np.concatenate([mO, mO], 1)),
            }
            in_maps.append(im)
    return in_maps


_PROGRAM_CACHE = {}


def get_program(cfg: Cfg):
    key = (cfg.N, cfg.D, cfg.H)
    if key not in _PROGRAM_CACHE:
        _PROGRAM_CACHE[key] = build_program(cfg)
    return _PROGRAM_CACHE[key]


def assemble(cfg: Cfg, results):
    c = cfg
    out = np.zeros((c.B, c.N, c.D), np.float32)
    for b in range(c.B):
        for s in range(2):
            o = results[2 * b + s]["out"]
            out[b, s * c.SQ : (s + 1) * c.SQ, :] = o.T
    return out


def kernel(**inputs) -> np.ndarray:
    cfg = Cfg(mini=False)
    nc = get_program(cfg)
    in_maps = host_prep(cfg, inputs)
    res = bass_utils.run_bass_kernel_spmd(
        nc, in_maps, core_ids=list(range(cfg.n_cores)), trace=False
    )
    return assemble(cfg, res.results)
